# revision 27
# baseline (speedup 1.0000x reference)
"""Trainium2 Bass kernel for nn_DPP: batched masked-Gram logdet minus shared
normalizer logdet.

out[i] = logdet(G * m_i m_i^T + diag(1-m_i)) - logdet(G + I),  G = B^T B

Sharding: data-parallel over the batch dim of x (one sample per NeuronCore,
B replicated). Host-side trick: each core gets B with its sample's SELECTED
columns permuted to the front, so the masked logdet is the logdet of the
LEADING ~n_sel block of the permuted Gram G' (logdet(G+I) is permutation
invariant), and one Gram serves both factorizations.

Device algorithm (per core):
  - G' = Bq^T Bq upper-triangle strips via fp8(e4m3) DoubleRow matmuls
    (fp32 PSUM accum, 2x PE throughput), B loaded in 16 chunked DMAs that
    overlap with the first Gram strips' accumulation chains.
  - Two interleaved left-looking blocked Cholesky factorizations (U-form,
    128-wide panels): A0 = leading-nblk-block masked G' (+ identity pad on
    partial blocks), A1 = G' + I.  A0's panels are OFFSET to pair with A1's
    tail panels.  Panels are software-pipelined: each panel's diagonal
    Schur chain is pre-accumulated (open PSUM group) during the PREVIOUS
    panel's refine, TRSM emits the diag+block1 tiles first so the next
    diagonal closes with a single matmul, and the remaining TRSM tiles /
    trailing accumulations / Gram strips fill the refine latency.
  - Each 128x128 diagonal pivot S is handled matmul-only ("refine" scheme):
      d = diag(S); r = 1/sqrt(d); q = r r^T
      X1 = striu(S) * q; X1T = stril(S) * q      (striu(DSD) = striu(S)*q)
      W = diag(r) (I - X1 + X1@X1)               (approx inv-chol factor)
      F = W^T S W - I                            (small)
      logdet(S) = sum(ln d) + tr F - tr F^2/2 + tr F^3/3
      What = W + W(-F/2 + 3F^2/8)                (What What^T ~ S^{-1})
    Panel: U_strip = What^T @ strip; trailing Schur updates use U (bf16).
    All ln d are batched into one ACT Ln at the end.
"""

import numpy as np
import ml_dtypes

P = 128
N = 2048           # matrix dim (= n columns of B)
NT = N // P        # 16 column tiles
NKT = 16           # contraction tiles (B rows padded 2000 -> 2048)
NKT2 = 8           # fp8 DoubleRow pairs
FT = 512           # free-dim tile for wide matmuls

_CACHE = {}
_last_in_maps = None
_PLAIN_GRAM = False  # CoreSim-only fallback (interp lacks 4D DoubleRow)


def _col_tiles(width_blocks, base_col, diag_first=False):
    """Gram tiling: split cols into <=512 tiles from the strip start."""
    tiles = []
    c = base_col
    end = base_col + width_blocks * P
    if diag_first:
        tiles.append((c, P))
        c += P
    while c < end:
        w = min(FT, end - c)
        tiles.append((c, w))
        c += w
    return tiles


def _panel_tiles(width_blocks, base_col):
    """Panel tiling: [diag P][block1 P][pad to abs 512 grid][512 grid...].
    block1 is split out so the next panel's diagonal Schur term needs only
    the first two TRSM tiles; the rest is 512-grid aligned."""
    end = base_col + width_blocks * P
    tiles = [(base_col, P)]
    c = base_col + P
    if c < end:
        tiles.append((c, P))
        c += P
    if c < end and c % FT:
        w = min(FT - c % FT, end - c)
        tiles.append((c, w))
        c += w
    while c < end:
        w = min(FT, end - c)
        tiles.append((c, w))
        c += w
    return tiles


def _build(nblk):
    import concourse.bass as bass
    import concourse.bacc as bacc
    import concourse.mybir as mybir
    from concourse.bass import ds, ts
    from concourse.masks import (
        make_identity,
        make_upper_triangular,
        make_lower_triangular,
    )
    from concourse.tile import TileContext
    from contextlib import ExitStack

    f32 = mybir.dt.float32
    bf16 = mybir.dt.bfloat16
    f8 = mybir.dt.float8e4
    AF = mybir.ActivationFunctionType
    OP = mybir.AluOpType
    PSUM = bass.MemorySpace.PSUM
    AX = mybir.AxisListType.X
    DR = mybir.MatmulPerfMode.DoubleRow
    OFF = NT - nblk  # m0 panel j runs at step t = j + OFF

    nc = bacc.Bacc()
    # B in block-pair layout [p, kt2, blk, slab, c] flattened per partition:
    # DoubleRow weights slices must be contiguous [P, 2, 128]
    bq = nc.dram_tensor("bq", [P, NKT2 * NT * 2 * P], f8, kind="ExternalInput")
    mcol_d = nc.dram_tensor("mcol", [P, nblk], f32, kind="ExternalInput")
    mrow_d = nc.dram_tensor("mrow", [P, nblk * P], bf16, kind="ExternalInput")
    out_d = nc.dram_tensor("out", [1, 1], f32, kind="ExternalOutput")

    with TileContext(nc) as tc, ExitStack() as stack:
        consts = stack.enter_context(tc.tile_pool(name="consts", bufs=1))
        I128 = consts.tile([P, P], f32, tag="i128")
        make_identity(nc, I128)
        I128b = consts.tile([P, P], bf16, tag="i128b")
        nc.vector.tensor_copy(I128b, I128)
        INEGB = consts.tile([P, P], bf16, tag="inegb")
        nc.vector.tensor_scalar(
            out=INEGB, in0=I128, scalar1=-1.0, scalar2=None, op0=OP.mult
        )
        STRIU = consts.tile([P, P], f32, tag="striu")
        make_upper_triangular(nc, STRIU, val=1.0, diag=False)
        STRIL = consts.tile([P, P], f32, tag="stril")
        make_lower_triangular(nc, STRIL, val=1.0, diag=False)
        STRIUN = consts.tile([P, P], f32, tag="striun")
        make_upper_triangular(nc, STRIUN, val=-1.0, diag=False)
        STRILN = consts.tile([P, P], f32, tag="striln")
        make_lower_triangular(nc, STRILN, val=-1.0, diag=False)
        INEGF = consts.tile([P, P], f32, tag="inegf")
        nc.vector.tensor_scalar(
            out=INEGF, in0=I128, scalar1=-1.0, scalar2=None, op0=OP.mult
        )
        mcol = consts.tile([P, nblk], f32, tag="mcol")
        nc.sync.dma_start(mcol, mcol_d[:, :])
        mrowrep = consts.tile([P, nblk * P], bf16, tag="mrowrep")
        nc.sync.dma_start(mrowrep, mrow_d[:, :])
        acc = consts.tile([P, 2], f32, tag="acc")
        nc.vector.memset(acc, 0.0)
        dstore = consts.tile([P, 2, NT], f32, tag="dstore")
        nc.vector.memset(dstore.rearrange("p a b -> p (a b)"), 1.0)
        onem_all = consts.tile([P, nblk], f32, tag="onem_all")
        nc.vector.tensor_scalar(
            out=onem_all, in0=mcol, scalar1=-1.0, scalar2=1.0,
            op0=OP.mult, op1=OP.add,
        )
        dfix_all = consts.tile([P, nblk, P], f32, tag="dfix_all")
        for i in range(nblk):
            nc.vector.tensor_scalar_mul(dfix_all[:, i, :], I128, onem_all[:, ds(i, 1)])

        gs = []  # gs[i]: [P, (NT-i)*P] bf16, absolute cols i*128..2048
        for i in range(NT):
            gs.append(consts.tile([P, (NT - i) * P], bf16, tag=f"gs{i}", name=f"gs{i}"))
        ub = {}  # panels of the two factorizations (m0: nblk-wide, m1: full)
        for i in range(nblk):
            ub[(0, i)] = consts.tile(
                [P, (nblk - i) * P], bf16, tag=f"ub0_{i}", name=f"ub0_{i}"
            )
        for i in range(NT):
            ub[(1, i)] = consts.tile(
                [P, (NT - i) * P], bf16, tag=f"ub1_{i}", name=f"ub1_{i}"
            )

        bpool = stack.enter_context(tc.tile_pool(name="bpool", bufs=1))
        gpsum = stack.enter_context(tc.tile_pool(name="gram_psum", bufs=2, space=PSUM))
        spool = stack.enter_context(tc.tile_pool(name="strip_pool", bufs=4))
        ppool = stack.enter_context(tc.tile_pool(name="pre_pool", bufs=6))
        rpool = stack.enter_context(tc.tile_pool(name="ref_pool", bufs=4))
        vpool = stack.enter_context(tc.tile_pool(name="vec_pool", bufs=4))
        apsum = stack.enter_context(tc.tile_pool(name="acc_psum", bufs=2, space=PSUM))
        wpsum = stack.enter_context(tc.tile_pool(name="work_psum", bufs=2, space=PSUM))
        dpsum = stack.enter_context(tc.tile_pool(name="diag_psum", bufs=1, space=PSUM))

        bt = bpool.tile([P, NKT2, NT, 2, P], f8, tag="bt")
        btf = bt.rearrange("p k b s c -> p (k b s c)")
        CH = NT * 2 * P  # one kt2 pair-slab chunk
        for kt in range(NKT2):
            nc.sync.dma_start(btf[:, ds(kt * CH, CH)], bq[:, ds(kt * CH, CH)])
        btr = bt.rearrange("p k b s c -> p k s b c")

        # round-robin engine pickers for balanced elementwise work
        _tt_state = 0
        _cp_state = 0

        def tt_eng():
            nonlocal _tt_state
            _tt_state += 1
            return (nc.vector, nc.gpsimd)[_tt_state % 2]

        def copy_rr(out, in_):
            nonlocal _cp_state
            _cp_state += 1
            if _cp_state % 2 == 0:
                nc.scalar.copy(out, in_)
            else:
                nc.vector.tensor_copy(out, in_)

        def gram_mm_chain(pt, i, c0, w, kt):
            if _PLAIN_GRAM:
                for s in range(2):
                    nc.tensor.matmul(
                        pt[:, :w],
                        bt[:, kt, i, s, :],
                        btr[:, kt, s, ds(c0 // P, w // P), :],
                        start=(kt == 0 and s == 0),
                        stop=(kt == NKT2 - 1 and s == 1),
                    )
                return
            nc.tensor.matmul(
                pt[:, :w],
                bt[:, kt, i, :, :],
                btr[:, kt, :, ds(c0 // P, w // P), :],
                start=(kt == 0),
                stop=(kt == NKT2 - 1),
                perf_mode=DR,
            )

        def gram_warmup():
            """Strips 0 and 1, kt-major across 6 concurrent PSUM chains so the
            Gram accumulation pipelines with the 16 chunked B DMAs."""
            chains = []
            pools = [(gpsum, "gp"), (gpsum, "gp"), (apsum, "ap"),
                     (apsum, "ap"), (wpsum, "w"), (wpsum, "w")]
            tiles01 = [(0, c0, w) for (c0, w) in _col_tiles(NT, 0)] + [
                (1, c0, w) for (c0, w) in _col_tiles(NT - 1, P)
            ]
            for (i, c0, w), (pool, tg) in zip(tiles01[:6], pools):
                chains.append((i, c0, w, pool.tile([P, FT], f32, tag=tg, name="gw")))
            for kt in range(NKT2):
                for (i, c0, w, pt) in chains:
                    gram_mm_chain(pt, i, c0, w, kt)
            for (i, c0, w, pt) in chains:
                nc.scalar.copy(gs[i][:, ds(c0 - i * P, w)], pt[:, :w])
                gram_done[(i, c0)] = True
            for (i, c0, w) in tiles01[6:]:
                pt = gpsum.tile([P, FT], f32, tag="gp", name="pt")
                for kt in range(NKT2):
                    gram_mm_chain(pt, i, c0, w, kt)
                nc.scalar.copy(gs[i][:, ds(c0 - i * P, w)], pt[:, :w])
                gram_done[(i, c0)] = True

        # ---------- software-pipeline state ----------
        from collections import deque

        gram_done = {}     # (strip, c0) -> emitted
        gramq = deque()
        tailq = {0: deque(), 1: deque()}
        emitted = {}       # (m, panel, tix) -> TRSM tile emitted
        pre_state = {0: None, 1: None}
        blk1_state = {0: None, 1: None}
        nblks_of = {0: nblk, 1: NT}
        _ptiles = {}
        for _i in range(nblk):
            _ptiles[(0, _i)] = _panel_tiles(nblk - _i, _i * P)
        for _i in range(NT):
            _ptiles[(1, _i)] = _panel_tiles(NT - _i, _i * P)

        def _gtile_of(i, col):
            return i * P + ((col - i * P) // FT) * FT

        def tile_ix_of(m, j, col):
            for tix, (c0, w) in enumerate(_ptiles[(m, j)]):
                if c0 <= col < c0 + w:
                    return tix
            raise AssertionError((m, j, col))

        def gram_gen(i):
            for (c0, w) in _col_tiles(NT - i, i * P):
                pt = gpsum.tile([P, FT], f32, tag="gp", name="pt")
                for kt in range(NKT2):
                    gram_mm_chain(pt, i, c0, w, kt)
                    if kt % 2 == 1 and kt < NKT2 - 1:
                        yield True
                nc.scalar.copy(gs[i][:, ds(c0 - i * P, w)], pt[:, :w])
                gram_done[(i, c0)] = True
                yield True

        def gram_master():
            if not gramq:
                return False
            g = gramq[0]
            try:
                return next(g)
            except StopIteration:
                gramq.popleft()
                return True

        def pull_gram_until(i, c0):
            guard = 0
            while (i, c0) not in gram_done:
                assert gramq, ("gram starved", i, c0)
                gram_master()
                guard += 1
                assert guard < 100000

        def tail_master(m):
            if not tailq[m]:
                return False
            g = tailq[m][0]
            try:
                return next(g)
            except StopIteration:
                tailq[m].popleft()
                return True

        def pull_tail_until(m, key):
            guard = 0
            while key not in emitted:
                assert tailq[m], ("tail starved", m, key)
                tail_master(m)
                guard += 1
                assert guard < 100000

        def diag_pre_vec(m, i):
            """Mask/I prep of gs diag block for panel i (no Schur terms)."""
            gsl = gs[i][:, ds(0, P)]
            pre = ppool.tile([P, P], bf16, tag=f"pre{m}", name="pre")
            if m == 1:
                nc.vector.tensor_add(pre, gsl, I128)
            else:
                tmp = ppool.tile([P, P], f32, tag="mtmp", name="mtmp")
                nc.gpsimd.tensor_mul(tmp, gsl, mrowrep[:, ds(i * P, P)])
                nc.gpsimd.tensor_scalar_mul(tmp, tmp, mcol[:, ds(i, 1)])
                nc.gpsimd.tensor_add(pre, tmp, dfix_all[:, i, :])
            return pre

        def diag_ap_mm(apt, m, n, j, start, stop=False):
            nc.tensor.matmul(
                apt,
                ub[(m, j)][:, ds((n - j) * P, P)],
                ub[(m, j)][:, ds((n - j) * P, P)],
                start=start,
                stop=stop,
            )

        def start_pre(m, n):
            if m == 0:
                return  # m0 chains are short; emitted force-closed at close
            pre_state[m] = {"n": n, "j": 0, "apt": None, "pre": None,
                            "started": False}

        def advance_pre(m, budget=6, force=False):
            st = pre_state[m]
            if st is None:
                return None
            n = st["n"]
            if st["pre"] is None:
                gc = _gtile_of(n, n * P)
                if (n, gc) not in gram_done:
                    if force:
                        pull_gram_until(n, gc)
                    else:
                        return False
                st["pre"] = diag_pre_vec(m, n)
                st["apt"] = dpsum.tile([P, P], f32, tag="dp", name="dp")
                nc.tensor.matmul(st["apt"], INEGB, st["pre"],
                                 start=True, stop=False)
                st["started"] = True
                budget -= 1
            while st["j"] <= n - 2:
                j = st["j"]
                key = (m, j, tile_ix_of(m, j, n * P))
                if key not in emitted:
                    if force:
                        pull_tail_until(m, key)
                    else:
                        return False
                diag_ap_mm(st["apt"], m, n, j, not st["started"])
                st["started"] = True
                st["j"] += 1
                budget -= 1
                if budget <= 0 and not force:
                    return True
            return None  # ready for close

        def close_diag(m, i, cx):
            sblk, sb = cx["sblk"], cx["sb"]
            if i == 0:
                gsl = gs[0][:, ds(0, P)]
                if m == 1:
                    nc.vector.tensor_add(sblk, gsl, I128)
                else:
                    tmp = ppool.tile([P, P], f32, tag="mtmp", name="mtmp")
                    nc.vector.tensor_mul(tmp, gsl, mrowrep[:, ds(0, P)])
                    nc.vector.tensor_scalar_mul(tmp, tmp, mcol[:, ds(0, 1)])
                    nc.vector.tensor_add(sblk, tmp, dfix_all[:, 0, :])
            elif m == 1:
                advance_pre(m, force=True)
                st = pre_state[m]
                assert st is not None and st["n"] == i
                diag_ap_mm(st["apt"], m, i, i - 1, not st["started"], stop=True)
                cx["sneg"] = st["apt"]
                nc.scalar.mul(sb, st["apt"], -1.0)
                pre_state[m] = None
                return
            else:
                gc = _gtile_of(i, i * P)
                pull_gram_until(i, gc)
                for j in range(i - 1):
                    pull_tail_until(0, (0, j, tile_ix_of(0, j, i * P)))
                pre = diag_pre_vec(0, i)
                apw = wpsum.tile([P, FT], f32, tag="w", name="apw")
                nc.tensor.matmul(apw[:, :P], INEGB, pre, start=True, stop=False)
                for j in range(i):
                    diag_ap_mm(apw[:, :P], 0, i, j, False, stop=(j == i - 1))
                cx["sneg"] = apw[:, :P]
                nc.scalar.mul(sb, apw[:, :P], -1.0)
                return
            (nc.scalar.copy if m == 1 else nc.gpsimd.tensor_copy)(sb, sblk)

        def new_panel(m, i):
            wblk = nblks_of[m] - i
            return {
                "i": i,
                "tiles": _ptiles[(m, i)],
                "strip": spool.tile([P, wblk * P], bf16, tag="strip", name="strip"),
                "sblk": rpool.tile([P, P], f32, tag="sblk", name="sblk"),
                "sb": rpool.tile([P, P], bf16, tag="sb", name="sb"),
                "sneg": None,
            }

        def start_blk1(m, i, cx):
            blk1_state[m] = None
            if len(cx["tiles"]) < 2:
                return
            if i == 0:
                if m == 0:
                    nc.vector.tensor_mul(
                        cx["strip"][:, ds(P, P)], gs[0][:, ds(P, P)],
                        mrowrep[:, ds(P, P)],
                    )
                return
            if m == 0:
                blk1_state[m] = {"i": i, "cx": cx, "forced": True}
                return
            blk1_state[m] = {"i": i, "cx": cx, "j": 0, "apt": None,
                             "pre1": None, "prepped": False, "started": False,
                             "forced": False}

        def advance_blk1(m, budget=6, force=False):
            st = blk1_state[m]
            if st is None or st.get("forced"):
                return None
            i = st["i"]
            cx = st["cx"]
            c0, _w = cx["tiles"][1]
            if not st["prepped"]:
                gc = _gtile_of(i, c0)
                if (i, gc) not in gram_done:
                    if force:
                        pull_gram_until(i, gc)
                    else:
                        return False
                if m == 0:
                    p1 = ppool.tile([P, P], f32, tag="p1", name="p1")
                    nc.gpsimd.tensor_mul(
                        p1, gs[i][:, ds(c0 - i * P, P)], mrowrep[:, ds(c0, P)]
                    )
                    st["pre1"] = p1
                st["apt"] = dpsum.tile([P, P], f32, tag="dp1", name="dp1")
                st["prepped"] = True
                budget -= 1
            while st["j"] <= i - 2:
                j = st["j"]
                k1 = (m, j, tile_ix_of(m, j, i * P))
                k2 = (m, j, tile_ix_of(m, j, c0))
                if k1 not in emitted or k2 not in emitted:
                    if force:
                        pull_tail_until(m, k1)
                        pull_tail_until(m, k2)
                    else:
                        return False
                nc.tensor.matmul(
                    st["apt"],
                    ub[(m, j)][:, ds((i - j) * P, P)],
                    ub[(m, j)][:, ds(c0 - j * P, P)],
                    start=not st["started"],
                    stop=False,
                )
                st["started"] = True
                st["j"] += 1
                budget -= 1
                if budget <= 0 and not force:
                    return True
            return None  # ready for blk1_finish

        def blk1_finish(m):
            st = blk1_state[m]
            if st is None:
                return
            i = st["i"]
            cx = st["cx"]
            c0, _w = cx["tiles"][1]
            dst = cx["strip"][:, ds(c0 - i * P, P)]
            if st.get("forced"):
                gc = _gtile_of(i, c0)
                pull_gram_until(i, gc)
                for j in range(i):
                    if j < i - 1:
                        pull_tail_until(m, (m, j, tile_ix_of(m, j, i * P)))
                    pull_tail_until(m, (m, j, tile_ix_of(m, j, c0)))
                p1 = ppool.tile([P, P], f32, tag="p1", name="p1")
                nc.gpsimd.tensor_mul(
                    p1, gs[i][:, ds(c0 - i * P, P)], mrowrep[:, ds(c0, P)]
                )
                apw = wpsum.tile([P, FT], f32, tag="w", name="apw")
                for j in range(i):
                    nc.tensor.matmul(
                        apw[:, :P],
                        ub[(m, j)][:, ds((i - j) * P, P)],
                        ub[(m, j)][:, ds(c0 - j * P, P)],
                        start=(j == 0),
                        stop=(j == i - 1),
                    )
                nc.vector.tensor_sub(dst, p1, apw[:, :P])
                blk1_state[m] = None
                return
            advance_blk1(m, force=True)
            key = (m, i - 1, tile_ix_of(m, i - 1, c0))
            pull_tail_until(m, key)
            nc.tensor.matmul(
                st["apt"],
                ub[(m, i - 1)][:, ds(P, P)],
                ub[(m, i - 1)][:, ds(c0 - (i - 1) * P, P)],
                start=not st["started"],
                stop=True,
            )
            nc.vector.tensor_sub(dst, gs[i][:, ds(c0 - i * P, P)], st["apt"])
            blk1_state[m] = None

        def refine_gen(m, i, cx):
            """Pivot-block factor; yields at cross-engine handoffs so filler
            matmuls can be emitted between dependent steps.  m1's elementwise
            ops ride DVE, m0's ride Pool (reduces are DVE-only)."""
            E = nc.vector if m == 1 else nc.gpsimd
            sb = cx["sb"]
            sneg = cx["sneg"]
            ssrc = sneg if sneg is not None else cx["sblk"]
            sgn = -1.0 if sneg is not None else 1.0
            dcol = dstore[:, m, ds(i, 1)]
            scr0 = rpool.tile([P, P], f32, tag="scr0", name="scr0")
            nc.vector.tensor_mul(scr0, ssrc, INEGF if sneg is not None else I128)
            nc.vector.tensor_reduce(dcol, scr0, AX, OP.add)
            rinv = vpool.tile([P, 1], f32, tag="rinv", name="rinv")
            nc.vector.reciprocal(rinv, dcol)
            yt = rpool.tile([P, P], f32, tag="yt", name="yt")
            nc.vector.tensor_mul(yt, ssrc, STRILN if sneg is not None else STRIL)
            yu = rpool.tile([P, P], f32, tag="yu", name="yu")
            nc.vector.tensor_mul(yu, ssrc, STRIUN if sneg is not None else STRIU)
            yield
            rt_ps = wpsum.tile([P, FT], f32, tag="w", name="rt_ps")
            nc.tensor.transpose(rt_ps[:1, :P], rinv, I128)
            rcol = vpool.tile([P, 1], f32, tag="rcol", name="rcol")
            nc.scalar.sqrt(rcol, rinv)
            rrow = vpool.tile([1, P], bf16, tag="rrow", name="rrow")
            nc.scalar.sqrt(rrow, rt_ps[:1, :P])
            yield
            q_ps = wpsum.tile([P, FT], f32, tag="w", name="q_ps")
            nc.tensor.matmul(q_ps[:, :P], rrow, rrow, start=True, stop=True)
            x1 = rpool.tile([P, P], bf16, tag="x1", name="x1")
            nc.vector.tensor_mul(x1, yu, q_ps[:, :P])
            x1t = rpool.tile([P, P], bf16, tag="x1t", name="x1t")
            nc.vector.tensor_mul(x1t, yt, q_ps[:, :P])
            yield
            # x2_ps accumulates X1@X1 - X1 + I entirely on PE
            x2_ps = wpsum.tile([P, FT], f32, tag="w", name="x2_ps")
            nc.tensor.matmul(x2_ps[:, :P], x1t, x1, start=True, stop=False)
            nc.tensor.matmul(x2_ps[:, :P], INEGB, x1, start=False, stop=False)
            nc.tensor.matmul(x2_ps[:, :P], I128b, I128b, start=False, stop=True)
            wfac = rpool.tile([P, P], bf16, tag="wfac", name="wfac")
            if m == 1:
                nc.vector.tensor_scalar_mul(wfac, x2_ps[:, :P], rcol)
            else:
                nc.scalar.activation(
                    wfac, x2_ps[:, :P], AF.Copy, scale=rcol)
            yield
            last = len(cx["tiles"]) == 1
            wt = None
            if not last:
                wt_ps = wpsum.tile([P, FT * 2], bf16, tag="w", name="wt_ps")
                nc.tensor.transpose(wt_ps[:, :P], wfac, I128b)
                wt = rpool.tile([P, P], bf16, tag="wt", name="wt")
                nc.scalar.copy(wt, wt_ps[:, :P])
            sw_ps = wpsum.tile([P, FT], f32, tag="w", name="sw_ps")
            nc.tensor.matmul(sw_ps[:, :P], sb, wfac, start=True, stop=True)
            swt = rpool.tile([P, P], bf16, tag="swt", name="swt")
            (nc.scalar.copy if m == 1 else nc.vector.tensor_copy)(
                swt, sw_ps[:, :P])
            yield
            # fpi_ps accumulates W^T S W - I on PE: result is F itself
            fpi_ps = wpsum.tile([P, FT], f32, tag="w", name="fpi_ps")
            nc.tensor.matmul(fpi_ps[:, :P], wfac, swt, start=True, stop=False)
            nc.tensor.matmul(fpi_ps[:, :P], INEGB, I128b, start=False, stop=True)
            ff = rpool.tile([P, P], bf16, tag="ff", name="ff")
            (nc.scalar.copy if m == 1 else nc.vector.tensor_copy)(
                ff, fpi_ps[:, :P])
            if not last:
                fs = rpool.tile([P, P], bf16, tag="fs", name="fs")
                E.tensor_scalar_mul(fs, ff, -0.5)
            yield
            if not last:
                wh_ps = wpsum.tile([P, FT], f32, tag="w", name="wh_ps")
                nc.tensor.matmul(wh_ps[:, :P], wt, fs, start=True, stop=True)
                what = rpool.tile([P, P], bf16, tag="what", name="what")
                nc.vector.tensor_add(what, wh_ps[:, :P], wfac)
                cx["what"] = what
                if m == 0:
                    whatm = rpool.tile([P, P], bf16, tag="whatm", name="whatm")
                    nc.gpsimd.tensor_scalar_mul(whatm, what, mcol[:, ds(i, 1)])
                    cx["whatm"] = whatm
            # logdet trace series, from bf16 copies, off the What chain
            trf = vpool.tile([P, 1], f32, tag="trf", name="trf")
            scr1 = rpool.tile([P, P], f32, tag="scr1", name="scr1")
            nc.vector.tensor_mul(scr1, ff, I128)
            nc.vector.tensor_reduce(trf, scr1, AX, OP.add)
            trf2 = vpool.tile([P, 1], f32, tag="trf2", name="trf2")
            scr2 = rpool.tile([P, P], f32, tag="scr2", name="scr2")
            nc.vector.tensor_mul(scr2, ff, ff)
            nc.vector.tensor_reduce(trf2, scr2, AX, OP.add)
            t1 = vpool.tile([P, 1], f32, tag="t1", name="t1")
            nc.vector.tensor_scalar(
                out=t1, in0=trf2, scalar1=-0.5, scalar2=None, op0=OP.mult
            )
            nc.vector.tensor_add(t1, t1, trf)
            nc.vector.tensor_add(acc[:, ds(m, 1)], acc[:, ds(m, 1)], t1)

        def trsm_tile(m, i, cx, tix):
            c0, w = cx["tiles"][tix]
            if m == 1 and i == 0 and tix > 0:
                rhs = gs[0][:, ds(c0, w)]
            elif tix == 0:
                rhs = cx["sb"]
            else:
                rhs = cx["strip"][:, ds(c0 - i * P, w)]
            lhs = cx["what"] if tix == 0 else cx.get("whatm", cx["what"])
            tp = wpsum.tile([P, FT], f32, tag="w", name="tp")
            nc.tensor.matmul(tp[:, :w], lhs, rhs, start=True, stop=True)
            dst = ub[(m, i)][:, ds(c0 - i * P, w)]
            if tix <= 1:
                nc.vector.tensor_copy(dst, tp[:, :w])
            else:
                copy_rr(dst, tp[:, :w])

        def trsm_head(m, i, cx):
            if len(cx["tiles"]) > 1:
                trsm_tile(m, i, cx, 1)
                emitted[(m, i, 1)] = True

        def tail_gen(m, i, cx):
            tiles = cx["tiles"]
            strip = cx["strip"]
            for tix in range(2, len(tiles)):
                c0, w = tiles[tix]
                if not (m == 1 and i == 0):
                    for col in (c0, c0 + w - 1):
                        gc = _gtile_of(i, col)
                        while (i, gc) not in gram_done:
                            yield False
                    ap = None
                    if i > 0:
                        ap = apsum.tile([P, FT], f32, tag="ap", name="ap")
                        for j in range(i):
                            nc.tensor.matmul(
                                ap[:, :w],
                                ub[(m, j)][:, ds((i - j) * P, P)],
                                ub[(m, j)][:, ds(c0 - j * P, w)],
                                start=(j == 0),
                                stop=(j == i - 1),
                            )
                            if j % 2 == 1 and j < i - 1:
                                yield True
                    gsl = gs[i][:, ds(c0 - i * P, w)]
                    dst = strip[:, ds(c0 - i * P, w)]
                    if m == 0:
                        if i > 0:
                            tmpm = spool.tile([P, FT], f32, tag="ptmp", name="tmpm")
                            tt_eng().tensor_mul(
                                tmpm[:, :w], gsl, mrowrep[:, ds(c0, w)]
                            )
                            nc.vector.tensor_sub(dst, tmpm[:, :w], ap[:, :w])
                        else:
                            tt_eng().tensor_mul(dst, gsl, mrowrep[:, ds(c0, w)])
                    else:
                        nc.vector.tensor_sub(dst, gsl, ap[:, :w])
                    yield True
                trsm_tile(m, i, cx, tix)
                emitted[(m, i, tix)] = True
                yield True

        # ---- software-pipelined emission --------------------------------
        gram_warmup()

        for t in range(NT):
            panels = [(1, t)]
            j0 = t - OFF
            if 0 <= j0 < nblk:
                panels.append((0, j0))
            cxs = {}
            gens = []
            live = []
            # m1 closes and primes its refine first; m0's boundary work then
            # lands in m1's early refine window (PE idle during sqrt/ttr).
            for (m, i) in panels:
                cxs[m] = new_panel(m, i)
                close_diag(m, i, cxs[m])
                start_blk1(m, i, cxs[m])
                g = refine_gen(m, i, cxs[m])
                next(g)
                gens.append(g)
                live.append(g)
            for (m, i) in panels:
                if i + 1 < nblks_of[m]:
                    start_pre(m, i + 1)
            if t + 2 < NT:
                gramq.append(gram_gen(t + 2))

            def mk(fn, *a):
                return lambda: fn(*a)

            tasks = [mk(tail_master, 1), mk(tail_master, 0), mk(gram_master)]
            for (m, i) in panels:
                if blk1_state[m] is not None:
                    tasks.append(mk(advance_blk1, m))
                if pre_state[m] is not None:
                    tasks.append(mk(advance_pre, m))
            fi = 0
            while live:
                for g in list(live):
                    try:
                        next(g)
                    except StopIteration:
                        live.remove(g)
                prog = 0
                attempts = 0
                while tasks and prog < 2 and attempts < 2 * len(tasks):
                    tk = tasks[fi % len(tasks)]
                    fi += 1
                    attempts += 1
                    r = tk()
                    if r is None:
                        tasks.remove(tk)
                    elif r:
                        prog += 1
            for (m, i) in panels:
                blk1_finish(m)
                trsm_head(m, i, cxs[m])
                if len(cxs[m]["tiles"]) > 2:
                    tailq[m].append(tail_gen(m, i, cxs[m]))

        guard = 0
        while tailq[0] or tailq[1] or gramq:
            p1 = tail_master(1)
            p0 = tail_master(0)
            pg = gram_master()
            guard = 0 if (p1 or p0 or pg) else guard + 1
            assert guard < 1000, "drain deadlock"

        # -------- final: batched Ln(d), partition-sum via matmul ------
        lnall = vpool.tile([P, 2, NT], f32, tag="lnall", name="lnall")
        nc.scalar.activation(
            lnall.rearrange("p a b -> p (a b)"),
            dstore.rearrange("p a b -> p (a b)"), AF.Ln,
        )
        ln0 = vpool.tile([P, 1], f32, tag="ln0", name="ln0")
        nc.vector.tensor_reduce(ln0, lnall[:, 0, :], AX, OP.add)
        ln1 = vpool.tile([P, 1], f32, tag="ln1", name="ln1")
        nc.vector.tensor_reduce(ln1, lnall[:, 1, :], AX, OP.add)
        accd = vpool.tile([P, 1], f32, tag="accd", name="accd")
        nc.vector.tensor_sub(accd, acc[:, 0:1], acc[:, 1:2])
        nc.vector.tensor_add(accd, accd, ln0)
        nc.vector.tensor_sub(accd, accd, ln1)
        ones = vpool.tile([P, 1], f32, tag="ones", name="ones")
        nc.vector.memset(ones, 1.0)
        r_ps = wpsum.tile([P, FT], f32, tag="w", name="r_ps")
        nc.tensor.matmul(r_ps[:1, :1], accd, ones, start=True, stop=True)
        res = vpool.tile([1, 1], f32, tag="res", name="res")
        nc.vector.tensor_copy(res, r_ps[:1, :1])
        nc.sync.dma_start(out_d[:, :], res)

    nc.finalize()
    return nc


def kernel(x, B):
    """Full inputs -> full output. x: [8, 2048] int32, B: [2000, 2048] f32."""
    from concourse.bass_utils import run_bass_kernel_spmd

    bs, n = x.shape
    k = B.shape[0]
    assert n == N and bs == 8

    B8 = B.astype(ml_dtypes.float8_e4m3fn)
    percore = []
    nblk = 1
    for c in range(bs):
        m = x[c] == 1
        sel = np.where(m)[0]
        unsel = np.where(~m)[0]
        n_c = len(sel)
        percore.append((np.concatenate([sel, unsel]), n_c))
        nblk = max(nblk, -(-n_c // P))

    if nblk not in _CACHE:
        _CACHE[nblk] = _build(nblk)
    nc = _CACHE[nblk]

    in_maps = []
    for perm, n_c in percore:
        bqc = np.zeros((N, N), dtype=ml_dtypes.float8_e4m3fn)
        bqc[:k, :] = B8[:, perm]
        # [p, kt2, blk, slab, c] layout, flattened per partition row
        bqc = np.ascontiguousarray(
            bqc.reshape(NKT2, 2, P, NT, P)
            .transpose(2, 0, 3, 1, 4)
            .reshape(P, NKT2 * NT * 2 * P)
        )
        idx = np.arange(nblk * P)
        mcol = np.ascontiguousarray(
            (idx.reshape(nblk, P).T < n_c).astype(np.float32)
        )
        mrow = np.ascontiguousarray(
            np.broadcast_to(
                (idx < n_c).astype(ml_dtypes.bfloat16), (P, nblk * P)
            )
        )
        in_maps.append({"bq": bqc, "mcol": mcol, "mrow": mrow})
    global _last_in_maps
    _last_in_maps = in_maps
    res = run_bass_kernel_spmd(nc, in_maps, core_ids=list(range(bs)))
    out = np.array([r["out"][0, 0] for r in res.results], dtype=np.float32)
    return out


# revision 32
# speedup vs baseline: 1.0210x; 1.0210x over previous
"""Trainium2 Bass kernel for nn_DPP: batched masked-Gram logdet minus shared
normalizer logdet.

out[i] = logdet(G * m_i m_i^T + diag(1-m_i)) - logdet(G + I),  G = B^T B

Sharding: data-parallel over the batch dim of x (one sample per NeuronCore,
B replicated). Host-side trick: each core gets B with its sample's SELECTED
columns permuted to the front, so the masked logdet is the logdet of the
LEADING ~n_sel block of the permuted Gram G' (logdet(G+I) is permutation
invariant), and one Gram serves both factorizations.

Device algorithm (per core):
  - G' = Bq^T Bq upper-triangle strips via fp8(e4m3) DoubleRow matmuls
    (fp32 PSUM accum, 2x PE throughput), B loaded in 16 chunked DMAs that
    overlap with the first Gram strips' accumulation chains.
  - Two interleaved left-looking blocked Cholesky factorizations (U-form,
    128-wide panels): A0 = leading-nblk-block masked G' (+ identity pad on
    partial blocks), A1 = G' + I.  A0's panels are OFFSET to pair with A1's
    tail panels.  Panels are software-pipelined: each panel's diagonal
    Schur chain is pre-accumulated (open PSUM group) during the PREVIOUS
    panel's refine, TRSM emits the diag+block1 tiles first so the next
    diagonal closes with a single matmul, and the remaining TRSM tiles /
    trailing accumulations / Gram strips fill the refine latency.
  - Each 128x128 diagonal pivot S is handled matmul-only ("refine" scheme):
      d = diag(S); r = 1/sqrt(d); q = r r^T
      X1 = striu(S) * q; X1T = stril(S) * q      (striu(DSD) = striu(S)*q)
      W = diag(r) (I - X1 + X1@X1)               (approx inv-chol factor)
      F = W^T S W - I                            (small)
      logdet(S) = sum(ln d) + tr F - tr F^2/2 + tr F^3/3
      What = W + W(-F/2 + 3F^2/8)                (What What^T ~ S^{-1})
    Panel: U_strip = What^T @ strip; trailing Schur updates use U (bf16).
    All ln d are batched into one ACT Ln at the end.
"""

import numpy as np
import ml_dtypes

P = 128
N = 2048           # matrix dim (= n columns of B)
NT = N // P        # 16 column tiles
NKT = 16           # contraction tiles (B rows padded 2000 -> 2048)
NKT2 = 8           # fp8 DoubleRow pairs
FT = 512           # free-dim tile for wide matmuls

_CACHE = {}
_last_in_maps = None
_PLAIN_GRAM = False  # CoreSim-only fallback (interp lacks 4D DoubleRow)


def _col_tiles(width_blocks, base_col, diag_first=False):
    """Gram tiling: split cols into <=512 tiles from the strip start."""
    tiles = []
    c = base_col
    end = base_col + width_blocks * P
    if diag_first:
        tiles.append((c, P))
        c += P
    while c < end:
        w = min(FT, end - c)
        tiles.append((c, w))
        c += w
    return tiles


def _panel_tiles(width_blocks, base_col):
    """Panel tiling: [diag P][block1 P][pad to abs 512 grid][512 grid...].
    block1 is split out so the next panel's diagonal Schur term needs only
    the first two TRSM tiles; the rest is 512-grid aligned."""
    end = base_col + width_blocks * P
    tiles = [(base_col, P)]
    c = base_col + P
    if c < end:
        tiles.append((c, P))
        c += P
    if c < end and c % FT:
        w = min(FT - c % FT, end - c)
        tiles.append((c, w))
        c += w
    while c < end:
        w = min(FT, end - c)
        tiles.append((c, w))
        c += w
    return tiles


def _build(nblk):
    import concourse.bass as bass
    import concourse.bacc as bacc
    import concourse.mybir as mybir
    from concourse.bass import ds, ts
    from concourse.masks import (
        make_identity,
        make_upper_triangular,
        make_lower_triangular,
    )
    from concourse.tile import TileContext
    from contextlib import ExitStack

    f32 = mybir.dt.float32
    bf16 = mybir.dt.bfloat16
    f8 = mybir.dt.float8e4
    AF = mybir.ActivationFunctionType
    OP = mybir.AluOpType
    PSUM = bass.MemorySpace.PSUM
    AX = mybir.AxisListType.X
    DR = mybir.MatmulPerfMode.DoubleRow
    OFF = NT - nblk  # m0 panel j runs at step t = j + OFF

    nc = bacc.Bacc()
    # B in block-pair layout [p, kt2, blk, slab, c] flattened per partition:
    # DoubleRow weights slices must be contiguous [P, 2, 128]
    bq = nc.dram_tensor("bq", [P, NKT2 * NT * 2 * P], f8, kind="ExternalInput")
    mcol_d = nc.dram_tensor("mcol", [P, nblk], f32, kind="ExternalInput")
    mrow_d = nc.dram_tensor("mrow", [P, nblk * P], bf16, kind="ExternalInput")
    out_d = nc.dram_tensor("out", [1, 1], f32, kind="ExternalOutput")

    with TileContext(nc) as tc, ExitStack() as stack:
        consts = stack.enter_context(tc.tile_pool(name="consts", bufs=1))
        I128 = consts.tile([P, P], f32, tag="i128")
        make_identity(nc, I128)
        I128b = consts.tile([P, P], bf16, tag="i128b")
        nc.vector.tensor_copy(I128b, I128)
        INEGB = consts.tile([P, P], bf16, tag="inegb")
        nc.vector.tensor_scalar(
            out=INEGB, in0=I128, scalar1=-1.0, scalar2=None, op0=OP.mult
        )
        STRIU = consts.tile([P, P], f32, tag="striu")
        make_upper_triangular(nc, STRIU, val=1.0, diag=False)
        STRIL = consts.tile([P, P], f32, tag="stril")
        make_lower_triangular(nc, STRIL, val=1.0, diag=False)
        STRIUN = consts.tile([P, P], f32, tag="striun")
        make_upper_triangular(nc, STRIUN, val=-1.0, diag=False)
        STRILN = consts.tile([P, P], f32, tag="striln")
        make_lower_triangular(nc, STRILN, val=-1.0, diag=False)
        INEGF = consts.tile([P, P], f32, tag="inegf")
        nc.vector.tensor_scalar(
            out=INEGF, in0=I128, scalar1=-1.0, scalar2=None, op0=OP.mult
        )
        mcol = consts.tile([P, nblk], f32, tag="mcol")
        nc.sync.dma_start(mcol, mcol_d[:, :])
        mrowrep = consts.tile([P, nblk * P], bf16, tag="mrowrep")
        nc.sync.dma_start(mrowrep, mrow_d[:, :])
        acc = consts.tile([P, 2], f32, tag="acc")
        nc.vector.memset(acc, 0.0)
        dstore = consts.tile([P, 2, NT], f32, tag="dstore")
        nc.vector.memset(dstore.rearrange("p a b -> p (a b)"), 1.0)
        onem_all = consts.tile([P, nblk], f32, tag="onem_all")
        nc.vector.tensor_scalar(
            out=onem_all, in0=mcol, scalar1=-1.0, scalar2=1.0,
            op0=OP.mult, op1=OP.add,
        )
        dfix_all = consts.tile([P, nblk, P], f32, tag="dfix_all")
        for i in range(nblk):
            nc.vector.tensor_scalar_mul(dfix_all[:, i, :], I128, onem_all[:, ds(i, 1)])

        gs = []  # gs[i]: [P, (NT-i)*P] bf16, absolute cols i*128..2048
        for i in range(NT):
            gs.append(consts.tile([P, (NT - i) * P], bf16, tag=f"gs{i}", name=f"gs{i}"))
        ub = {}  # panels of the two factorizations (m0: nblk-wide, m1: full)
        for i in range(nblk):
            ub[(0, i)] = consts.tile(
                [P, (nblk - i) * P], bf16, tag=f"ub0_{i}", name=f"ub0_{i}"
            )
        for i in range(NT):
            ub[(1, i)] = consts.tile(
                [P, (NT - i) * P], bf16, tag=f"ub1_{i}", name=f"ub1_{i}"
            )

        bpool = stack.enter_context(tc.tile_pool(name="bpool", bufs=1))
        gpsum = stack.enter_context(tc.tile_pool(name="gram_psum", bufs=2, space=PSUM))
        spool = stack.enter_context(tc.tile_pool(name="strip_pool", bufs=4))
        ppool = stack.enter_context(tc.tile_pool(name="pre_pool", bufs=6))
        rpool = stack.enter_context(tc.tile_pool(name="ref_pool", bufs=4))
        vpool = stack.enter_context(tc.tile_pool(name="vec_pool", bufs=4))
        apsum = stack.enter_context(tc.tile_pool(name="acc_psum", bufs=2, space=PSUM))
        wpsum = stack.enter_context(tc.tile_pool(name="work_psum", bufs=2, space=PSUM))
        dpsum = stack.enter_context(tc.tile_pool(name="diag_psum", bufs=1, space=PSUM))

        bt = bpool.tile([P, NKT2, NT, 2, P], f8, tag="bt")
        btf = bt.rearrange("p k b s c -> p (k b s c)")
        CH = NT * 2 * P  # one kt2 pair-slab chunk
        for kt in range(NKT2):
            nc.sync.dma_start(btf[:, ds(kt * CH, CH)], bq[:, ds(kt * CH, CH)])
        btr = bt.rearrange("p k b s c -> p k s b c")

        # round-robin engine pickers for balanced elementwise work
        _tt_state = 0
        _cp_state = 0

        def tt_eng():
            nonlocal _tt_state
            _tt_state += 1
            return (nc.vector, nc.gpsimd)[_tt_state % 2]

        def copy_rr(out, in_):
            nonlocal _cp_state
            _cp_state += 1
            if _cp_state % 2 == 0:
                nc.scalar.copy(out, in_)
            else:
                nc.vector.tensor_copy(out, in_)

        def gram_mm_chain(pt, i, c0, w, kt):
            if _PLAIN_GRAM:
                for s in range(2):
                    nc.tensor.matmul(
                        pt[:, :w],
                        bt[:, kt, i, s, :],
                        btr[:, kt, s, ds(c0 // P, w // P), :],
                        start=(kt == 0 and s == 0),
                        stop=(kt == NKT2 - 1 and s == 1),
                    )
                return
            nc.tensor.matmul(
                pt[:, :w],
                bt[:, kt, i, :, :],
                btr[:, kt, :, ds(c0 // P, w // P), :],
                start=(kt == 0),
                stop=(kt == NKT2 - 1),
                perf_mode=DR,
            )

        def gram_warmup():
            """Strips 0 and 1, kt-major across 6 concurrent PSUM chains so the
            Gram accumulation pipelines with the 16 chunked B DMAs."""
            chains = []
            pools = [(gpsum, "gp"), (gpsum, "gp"), (apsum, "ap"),
                     (apsum, "ap"), (wpsum, "w"), (wpsum, "w")]
            tiles01 = [(0, c0, w) for (c0, w) in _col_tiles(NT, 0)] + [
                (1, c0, w) for (c0, w) in _col_tiles(NT - 1, P)
            ]
            for (i, c0, w), (pool, tg) in zip(tiles01[:6], pools):
                chains.append((i, c0, w, pool.tile([P, FT], f32, tag=tg, name="gw")))
            for kt in range(NKT2):
                for (i, c0, w, pt) in chains:
                    gram_mm_chain(pt, i, c0, w, kt)
            for (i, c0, w, pt) in chains:
                nc.scalar.copy(gs[i][:, ds(c0 - i * P, w)], pt[:, :w])
                gram_done[(i, c0)] = True
            for (i, c0, w) in tiles01[6:]:
                pt = gpsum.tile([P, FT], f32, tag="gp", name="pt")
                for kt in range(NKT2):
                    gram_mm_chain(pt, i, c0, w, kt)
                nc.scalar.copy(gs[i][:, ds(c0 - i * P, w)], pt[:, :w])
                gram_done[(i, c0)] = True

        # ---------- software-pipeline state ----------
        from collections import deque

        gram_done = {}     # (strip, c0) -> emitted
        gramq = deque()
        tailq = {0: deque(), 1: deque()}
        emitted = {}       # (m, panel, tix) -> TRSM tile emitted
        pre_state = {0: None, 1: None}
        blk1_state = {0: None, 1: None}
        nblks_of = {0: nblk, 1: NT}
        _ptiles = {}
        for _i in range(nblk):
            _ptiles[(0, _i)] = _panel_tiles(nblk - _i, _i * P)
        for _i in range(NT):
            _ptiles[(1, _i)] = _panel_tiles(NT - _i, _i * P)

        def _gtile_of(i, col):
            return i * P + ((col - i * P) // FT) * FT

        def tile_ix_of(m, j, col):
            for tix, (c0, w) in enumerate(_ptiles[(m, j)]):
                if c0 <= col < c0 + w:
                    return tix
            raise AssertionError((m, j, col))

        def gram_gen(i):
            for (c0, w) in _col_tiles(NT - i, i * P):
                pt = gpsum.tile([P, FT], f32, tag="gp", name="pt")
                for kt in range(NKT2):
                    gram_mm_chain(pt, i, c0, w, kt)
                    if kt % 2 == 1 and kt < NKT2 - 1:
                        yield True
                nc.scalar.copy(gs[i][:, ds(c0 - i * P, w)], pt[:, :w])
                gram_done[(i, c0)] = True
                yield True

        def gram_master():
            if not gramq:
                return False
            g = gramq[0]
            try:
                return next(g)
            except StopIteration:
                gramq.popleft()
                return True

        def pull_gram_until(i, c0):
            guard = 0
            while (i, c0) not in gram_done:
                assert gramq, ("gram starved", i, c0)
                gram_master()
                guard += 1
                assert guard < 100000

        def tail_master(m):
            if not tailq[m]:
                return False
            g = tailq[m][0]
            try:
                return next(g)
            except StopIteration:
                tailq[m].popleft()
                return True

        def pull_tail_until(m, key):
            guard = 0
            while key not in emitted:
                assert tailq[m], ("tail starved", m, key)
                tail_master(m)
                guard += 1
                assert guard < 100000

        def diag_pre_vec(m, i):
            """Mask/I prep of gs diag block for panel i (no Schur terms)."""
            gsl = gs[i][:, ds(0, P)]
            pre = ppool.tile([P, P], bf16, tag=f"pre{m}", name="pre")
            if m == 1:
                nc.vector.tensor_add(pre, gsl, I128)
            else:
                tmp = ppool.tile([P, P], f32, tag="mtmp", name="mtmp")
                nc.gpsimd.tensor_mul(tmp, gsl, mrowrep[:, ds(i * P, P)])
                nc.gpsimd.tensor_scalar_mul(tmp, tmp, mcol[:, ds(i, 1)])
                nc.gpsimd.tensor_add(pre, tmp, dfix_all[:, i, :])
            return pre

        def diag_ap_mm(apt, m, n, j, start, stop=False):
            nc.tensor.matmul(
                apt,
                ub[(m, j)][:, ds((n - j) * P, P)],
                ub[(m, j)][:, ds((n - j) * P, P)],
                start=start,
                stop=stop,
            )

        def start_pre(m, n):
            if m == 0:
                return  # m0 chains are short; emitted force-closed at close
            pre_state[m] = {"n": n, "j": 0, "apt": None, "pre": None,
                            "started": False}

        def advance_pre(m, budget=6, force=False):
            st = pre_state[m]
            if st is None:
                return None
            n = st["n"]
            if st["pre"] is None:
                gc = _gtile_of(n, n * P)
                if (n, gc) not in gram_done:
                    if force:
                        pull_gram_until(n, gc)
                    else:
                        return False
                st["pre"] = diag_pre_vec(m, n)
                st["apt"] = dpsum.tile([P, P], f32, tag="dp", name="dp")
                nc.tensor.matmul(st["apt"], INEGB, st["pre"],
                                 start=True, stop=False)
                st["started"] = True
                budget -= 1
            while st["j"] <= n - 2:
                j = st["j"]
                key = (m, j, tile_ix_of(m, j, n * P))
                if key not in emitted:
                    if force:
                        pull_tail_until(m, key)
                    else:
                        return False
                diag_ap_mm(st["apt"], m, n, j, not st["started"])
                st["started"] = True
                st["j"] += 1
                budget -= 1
                if budget <= 0 and not force:
                    return True
            return None  # ready for close

        def close_diag(m, i, cx):
            sblk, sb = cx["sblk"], cx["sb"]
            if i == 0:
                gsl = gs[0][:, ds(0, P)]
                if m == 1:
                    nc.vector.tensor_add(sblk, gsl, I128)
                else:
                    tmp = ppool.tile([P, P], f32, tag="mtmp", name="mtmp")
                    nc.vector.tensor_mul(tmp, gsl, mrowrep[:, ds(0, P)])
                    nc.vector.tensor_scalar_mul(tmp, tmp, mcol[:, ds(0, 1)])
                    nc.vector.tensor_add(sblk, tmp, dfix_all[:, 0, :])
            elif m == 1:
                advance_pre(m, force=True)
                st = pre_state[m]
                assert st is not None and st["n"] == i
                diag_ap_mm(st["apt"], m, i, i - 1, not st["started"], stop=True)
                cx["sneg"] = st["apt"]
                nc.scalar.mul(sb, st["apt"], -1.0)
                pre_state[m] = None
                return
            else:
                gc = _gtile_of(i, i * P)
                pull_gram_until(i, gc)
                for j in range(i - 1):
                    pull_tail_until(0, (0, j, tile_ix_of(0, j, i * P)))
                pre = diag_pre_vec(0, i)
                apw = wpsum.tile([P, FT], f32, tag="w", name="apw")
                nc.tensor.matmul(apw[:, :P], INEGB, pre, start=True, stop=False)
                for j in range(i):
                    diag_ap_mm(apw[:, :P], 0, i, j, False, stop=(j == i - 1))
                cx["sneg"] = apw[:, :P]
                nc.scalar.mul(sb, apw[:, :P], -1.0)
                return
            (nc.scalar.copy if m == 1 else nc.gpsimd.tensor_copy)(sb, sblk)

        def new_panel(m, i):
            wblk = nblks_of[m] - i
            return {
                "i": i,
                "tiles": _ptiles[(m, i)],
                "strip": spool.tile([P, wblk * P], bf16, tag="strip", name="strip"),
                "sblk": rpool.tile([P, P], f32, tag="sblk", name="sblk"),
                "sb": rpool.tile([P, P], bf16, tag="sb", name="sb"),
                "sneg": None,
            }

        def start_blk1(m, i, cx):
            blk1_state[m] = None
            if len(cx["tiles"]) < 2:
                return
            if i == 0:
                if m == 0:
                    nc.vector.tensor_mul(
                        cx["strip"][:, ds(P, P)], gs[0][:, ds(P, P)],
                        mrowrep[:, ds(P, P)],
                    )
                return
            if m == 0:
                blk1_state[m] = {"i": i, "cx": cx, "forced": True}
                return
            blk1_state[m] = {"i": i, "cx": cx, "j": 0, "apt": None,
                             "pre1": None, "prepped": False, "started": False,
                             "forced": False}

        def advance_blk1(m, budget=6, force=False):
            st = blk1_state[m]
            if st is None or st.get("forced"):
                return None
            i = st["i"]
            cx = st["cx"]
            c0, _w = cx["tiles"][1]
            if not st["prepped"]:
                gc = _gtile_of(i, c0)
                if (i, gc) not in gram_done:
                    if force:
                        pull_gram_until(i, gc)
                    else:
                        return False
                if m == 0:
                    p1 = ppool.tile([P, P], f32, tag="p1", name="p1")
                    nc.gpsimd.tensor_mul(
                        p1, gs[i][:, ds(c0 - i * P, P)], mrowrep[:, ds(c0, P)]
                    )
                    st["pre1"] = p1
                st["apt"] = dpsum.tile([P, P], f32, tag="dp1", name="dp1")
                st["prepped"] = True
                budget -= 1
            while st["j"] <= i - 2:
                j = st["j"]
                k1 = (m, j, tile_ix_of(m, j, i * P))
                k2 = (m, j, tile_ix_of(m, j, c0))
                if k1 not in emitted or k2 not in emitted:
                    if force:
                        pull_tail_until(m, k1)
                        pull_tail_until(m, k2)
                    else:
                        return False
                nc.tensor.matmul(
                    st["apt"],
                    ub[(m, j)][:, ds((i - j) * P, P)],
                    ub[(m, j)][:, ds(c0 - j * P, P)],
                    start=not st["started"],
                    stop=False,
                )
                st["started"] = True
                st["j"] += 1
                budget -= 1
                if budget <= 0 and not force:
                    return True
            return None  # ready for blk1_finish

        def blk1_finish(m):
            st = blk1_state[m]
            if st is None:
                return
            i = st["i"]
            cx = st["cx"]
            c0, _w = cx["tiles"][1]
            dst = cx["strip"][:, ds(c0 - i * P, P)]
            if st.get("forced"):
                gc = _gtile_of(i, c0)
                pull_gram_until(i, gc)
                for j in range(i):
                    if j < i - 1:
                        pull_tail_until(m, (m, j, tile_ix_of(m, j, i * P)))
                    pull_tail_until(m, (m, j, tile_ix_of(m, j, c0)))
                p1 = ppool.tile([P, P], f32, tag="p1", name="p1")
                nc.gpsimd.tensor_mul(
                    p1, gs[i][:, ds(c0 - i * P, P)], mrowrep[:, ds(c0, P)]
                )
                apw = wpsum.tile([P, FT], f32, tag="w", name="apw")
                for j in range(i):
                    nc.tensor.matmul(
                        apw[:, :P],
                        ub[(m, j)][:, ds((i - j) * P, P)],
                        ub[(m, j)][:, ds(c0 - j * P, P)],
                        start=(j == 0),
                        stop=(j == i - 1),
                    )
                nc.vector.tensor_sub(dst, p1, apw[:, :P])
                blk1_state[m] = None
                return
            advance_blk1(m, force=True)
            key = (m, i - 1, tile_ix_of(m, i - 1, c0))
            pull_tail_until(m, key)
            nc.tensor.matmul(
                st["apt"],
                ub[(m, i - 1)][:, ds(P, P)],
                ub[(m, i - 1)][:, ds(c0 - (i - 1) * P, P)],
                start=not st["started"],
                stop=True,
            )
            nc.vector.tensor_sub(dst, gs[i][:, ds(c0 - i * P, P)], st["apt"])
            blk1_state[m] = None

        def refine_gen(m, i, cx):
            """Pivot-block factor; yields at cross-engine handoffs so filler
            matmuls can be emitted between dependent steps.  m1's elementwise
            ops ride DVE, m0's ride Pool (reduces are DVE-only)."""
            E = nc.vector if m == 1 else nc.gpsimd
            sb = cx["sb"]
            sneg = cx["sneg"]
            ssrc = sneg if sneg is not None else cx["sblk"]
            sgn = -1.0 if sneg is not None else 1.0
            dcol = dstore[:, m, ds(i, 1)]
            scr0 = rpool.tile([P, P], f32, tag="scr0", name="scr0")
            nc.vector.tensor_mul(scr0, ssrc, INEGF if sneg is not None else I128)
            nc.vector.tensor_reduce(dcol, scr0, AX, OP.add)
            rinv = vpool.tile([P, 1], f32, tag="rinv", name="rinv")
            nc.vector.reciprocal(rinv, dcol)
            yt = rpool.tile([P, P], f32, tag="yt", name="yt")
            nc.vector.tensor_mul(yt, ssrc, STRILN if sneg is not None else STRIL)
            yu = rpool.tile([P, P], f32, tag="yu", name="yu")
            nc.vector.tensor_mul(yu, ssrc, STRIUN if sneg is not None else STRIU)
            yield
            rt_ps = wpsum.tile([P, FT], f32, tag="w", name="rt_ps")
            nc.tensor.transpose(rt_ps[:1, :P], rinv, I128)
            rcol = vpool.tile([P, 1], f32, tag="rcol", name="rcol")
            nc.scalar.sqrt(rcol, rinv)
            rrow = vpool.tile([1, P], bf16, tag="rrow", name="rrow")
            nc.scalar.sqrt(rrow, rt_ps[:1, :P])
            yield
            q_ps = wpsum.tile([P, FT], f32, tag="w", name="q_ps")
            nc.tensor.matmul(q_ps[:, :P], rrow, rrow, start=True, stop=True)
            x1 = rpool.tile([P, P], bf16, tag="x1", name="x1")
            nc.vector.tensor_mul(x1, yu, q_ps[:, :P])
            x1t = rpool.tile([P, P], bf16, tag="x1t", name="x1t")
            nc.vector.tensor_mul(x1t, yt, q_ps[:, :P])
            yield
            # x2_ps accumulates X1@X1 - X1 + I entirely on PE
            x2_ps = wpsum.tile([P, FT], f32, tag="w", name="x2_ps")
            nc.tensor.matmul(x2_ps[:, :P], x1t, x1, start=True, stop=False)
            nc.tensor.matmul(x2_ps[:, :P], INEGB, x1, start=False, stop=False)
            nc.tensor.matmul(x2_ps[:, :P], I128b, I128b, start=False, stop=True)
            wfac = rpool.tile([P, P], bf16, tag="wfac", name="wfac")
            if m == 1:
                nc.vector.tensor_scalar_mul(wfac, x2_ps[:, :P], rcol)
            else:
                nc.scalar.activation(
                    wfac, x2_ps[:, :P], AF.Copy, scale=rcol)
            yield
            last = len(cx["tiles"]) == 1
            wt = None
            if not last:
                wt_ps = wpsum.tile([P, FT * 2], bf16, tag="w", name="wt_ps")
                nc.tensor.transpose(wt_ps[:, :P], wfac, I128b)
                wt = rpool.tile([P, P], bf16, tag="wt", name="wt")
                nc.scalar.copy(wt, wt_ps[:, :P])
            sw_ps = wpsum.tile([P, FT], f32, tag="w", name="sw_ps")
            nc.tensor.matmul(sw_ps[:, :P], sb, wfac, start=True, stop=True)
            swt = rpool.tile([P, P], bf16, tag="swt", name="swt")
            nc.scalar.copy(swt, sw_ps[:, :P])
            yield
            # fpi_ps accumulates W^T S W - I on PE: result is F itself
            fpi_ps = wpsum.tile([P, FT], f32, tag="w", name="fpi_ps")
            nc.tensor.matmul(fpi_ps[:, :P], wfac, swt, start=True, stop=False)
            nc.tensor.matmul(fpi_ps[:, :P], INEGB, I128b, start=False, stop=True)
            ff = rpool.tile([P, P], bf16, tag="ff", name="ff")
            nc.scalar.copy(ff, fpi_ps[:, :P])
            if not last:
                fs = rpool.tile([P, P], bf16, tag="fs", name="fs")
                E.tensor_scalar_mul(fs, ff, -0.5)
            yield
            if not last:
                wh_ps = wpsum.tile([P, FT], f32, tag="w", name="wh_ps")
                nc.tensor.matmul(wh_ps[:, :P], wt, fs, start=True, stop=True)
                what = rpool.tile([P, P], bf16, tag="what", name="what")
                nc.vector.tensor_add(what, wh_ps[:, :P], wfac)
                cx["what"] = what
                if m == 0:
                    whatm = rpool.tile([P, P], bf16, tag="whatm", name="whatm")
                    nc.gpsimd.tensor_scalar_mul(whatm, what, mcol[:, ds(i, 1)])
                    cx["whatm"] = whatm
            # logdet trace series, from bf16 copies, off the What chain
            trf = vpool.tile([P, 1], f32, tag="trf", name="trf")
            scr1 = rpool.tile([P, P], f32, tag="scr1", name="scr1")
            nc.gpsimd.tensor_mul(scr1, ff, I128)
            nc.vector.tensor_reduce(trf, scr1, AX, OP.add)
            trf2 = vpool.tile([P, 1], f32, tag="trf2", name="trf2")
            scr2 = rpool.tile([P, P], f32, tag="scr2", name="scr2")
            nc.gpsimd.tensor_mul(scr2, ff, ff)
            nc.vector.tensor_reduce(trf2, scr2, AX, OP.add)
            t1 = vpool.tile([P, 1], f32, tag="t1", name="t1")
            nc.vector.tensor_scalar(
                out=t1, in0=trf2, scalar1=-0.5, scalar2=None, op0=OP.mult
            )
            nc.vector.tensor_add(t1, t1, trf)
            nc.vector.tensor_add(acc[:, ds(m, 1)], acc[:, ds(m, 1)], t1)

        def trsm_tile(m, i, cx, tix):
            c0, w = cx["tiles"][tix]
            if m == 1 and i == 0 and tix > 0:
                rhs = gs[0][:, ds(c0, w)]
            elif tix == 0:
                rhs = cx["sb"]
            else:
                rhs = cx["strip"][:, ds(c0 - i * P, w)]
            lhs = cx["what"] if tix == 0 else cx.get("whatm", cx["what"])
            tp = wpsum.tile([P, FT], f32, tag="w", name="tp")
            nc.tensor.matmul(tp[:, :w], lhs, rhs, start=True, stop=True)
            dst = ub[(m, i)][:, ds(c0 - i * P, w)]
            if tix <= 1:
                nc.vector.tensor_copy(dst, tp[:, :w])
            else:
                copy_rr(dst, tp[:, :w])

        def trsm_head(m, i, cx):
            if len(cx["tiles"]) > 1:
                trsm_tile(m, i, cx, 1)
                emitted[(m, i, 1)] = True

        def tail_gen(m, i, cx):
            tiles = cx["tiles"]
            strip = cx["strip"]
            for tix in range(2, len(tiles)):
                c0, w = tiles[tix]
                if not (m == 1 and i == 0):
                    for col in (c0, c0 + w - 1):
                        gc = _gtile_of(i, col)
                        while (i, gc) not in gram_done:
                            yield False
                    ap = None
                    if i > 0:
                        ap = apsum.tile([P, FT], f32, tag="ap", name="ap")
                        for j in range(i):
                            nc.tensor.matmul(
                                ap[:, :w],
                                ub[(m, j)][:, ds((i - j) * P, P)],
                                ub[(m, j)][:, ds(c0 - j * P, w)],
                                start=(j == 0),
                                stop=(j == i - 1),
                            )
                            if j % 2 == 1 and j < i - 1:
                                yield True
                    gsl = gs[i][:, ds(c0 - i * P, w)]
                    dst = strip[:, ds(c0 - i * P, w)]
                    if m == 0:
                        if i > 0:
                            tmpm = spool.tile([P, FT], f32, tag="ptmp", name="tmpm")
                            tt_eng().tensor_mul(
                                tmpm[:, :w], gsl, mrowrep[:, ds(c0, w)]
                            )
                            nc.vector.tensor_sub(dst, tmpm[:, :w], ap[:, :w])
                        else:
                            tt_eng().tensor_mul(dst, gsl, mrowrep[:, ds(c0, w)])
                    else:
                        nc.vector.tensor_sub(dst, gsl, ap[:, :w])
                    yield True
                trsm_tile(m, i, cx, tix)
                emitted[(m, i, tix)] = True
                yield True

        # ---- software-pipelined emission --------------------------------
        gram_warmup()

        for t in range(NT):
            panels = [(1, t)]
            j0 = t - OFF
            if 0 <= j0 < nblk:
                panels.append((0, j0))
            cxs = {}
            gens = []
            live = []
            # m1 closes and primes its refine first; m0's boundary work then
            # lands in m1's early refine window (PE idle during sqrt/ttr).
            for (m, i) in panels:
                cxs[m] = new_panel(m, i)
                close_diag(m, i, cxs[m])
                start_blk1(m, i, cxs[m])
                g = refine_gen(m, i, cxs[m])
                next(g)
                gens.append(g)
                live.append(g)
            for (m, i) in panels:
                if i + 1 < nblks_of[m]:
                    start_pre(m, i + 1)
            if t + 2 < NT:
                gramq.append(gram_gen(t + 2))

            def mk(fn, *a):
                return lambda: fn(*a)

            tasks = [mk(tail_master, 1), mk(tail_master, 0), mk(gram_master)]
            for (m, i) in panels:
                if blk1_state[m] is not None:
                    tasks.append(mk(advance_blk1, m))
                if pre_state[m] is not None:
                    tasks.append(mk(advance_pre, m))
            fi = 0
            while live:
                for g in list(live):
                    try:
                        next(g)
                    except StopIteration:
                        live.remove(g)
                prog = 0
                attempts = 0
                while tasks and prog < 3 and attempts < 2 * len(tasks):
                    tk = tasks[fi % len(tasks)]
                    fi += 1
                    attempts += 1
                    r = tk()
                    if r is None:
                        tasks.remove(tk)
                    elif r:
                        prog += 1
            for (m, i) in panels:
                blk1_finish(m)
                trsm_head(m, i, cxs[m])
                if len(cxs[m]["tiles"]) > 2:
                    tailq[m].append(tail_gen(m, i, cxs[m]))

        guard = 0
        while tailq[0] or tailq[1] or gramq:
            p1 = tail_master(1)
            p0 = tail_master(0)
            pg = gram_master()
            guard = 0 if (p1 or p0 or pg) else guard + 1
            assert guard < 1000, "drain deadlock"

        # -------- final: batched Ln(d), partition-sum via matmul ------
        lnall = vpool.tile([P, 2, NT], f32, tag="lnall", name="lnall")
        nc.scalar.activation(
            lnall.rearrange("p a b -> p (a b)"),
            dstore.rearrange("p a b -> p (a b)"), AF.Ln,
        )
        ln0 = vpool.tile([P, 1], f32, tag="ln0", name="ln0")
        nc.vector.tensor_reduce(ln0, lnall[:, 0, :], AX, OP.add)
        ln1 = vpool.tile([P, 1], f32, tag="ln1", name="ln1")
        nc.vector.tensor_reduce(ln1, lnall[:, 1, :], AX, OP.add)
        accd = vpool.tile([P, 1], f32, tag="accd", name="accd")
        nc.vector.tensor_sub(accd, acc[:, 0:1], acc[:, 1:2])
        nc.vector.tensor_add(accd, accd, ln0)
        nc.vector.tensor_sub(accd, accd, ln1)
        ones = vpool.tile([P, 1], f32, tag="ones", name="ones")
        nc.vector.memset(ones, 1.0)
        r_ps = wpsum.tile([P, FT], f32, tag="w", name="r_ps")
        nc.tensor.matmul(r_ps[:1, :1], accd, ones, start=True, stop=True)
        res = vpool.tile([1, 1], f32, tag="res", name="res")
        nc.vector.tensor_copy(res, r_ps[:1, :1])
        nc.sync.dma_start(out_d[:, :], res)

    nc.finalize()
    return nc


def kernel(x, B):
    """Full inputs -> full output. x: [8, 2048] int32, B: [2000, 2048] f32."""
    from concourse.bass_utils import run_bass_kernel_spmd

    bs, n = x.shape
    k = B.shape[0]
    assert n == N and bs == 8

    B8 = B.astype(ml_dtypes.float8_e4m3fn)
    percore = []
    nblk = 1
    for c in range(bs):
        m = x[c] == 1
        sel = np.where(m)[0]
        unsel = np.where(~m)[0]
        n_c = len(sel)
        percore.append((np.concatenate([sel, unsel]), n_c))
        nblk = max(nblk, -(-n_c // P))

    if nblk not in _CACHE:
        _CACHE[nblk] = _build(nblk)
    nc = _CACHE[nblk]

    in_maps = []
    for perm, n_c in percore:
        bqc = np.zeros((N, N), dtype=ml_dtypes.float8_e4m3fn)
        bqc[:k, :] = B8[:, perm]
        # [p, kt2, blk, slab, c] layout, flattened per partition row
        bqc = np.ascontiguousarray(
            bqc.reshape(NKT2, 2, P, NT, P)
            .transpose(2, 0, 3, 1, 4)
            .reshape(P, NKT2 * NT * 2 * P)
        )
        idx = np.arange(nblk * P)
        mcol = np.ascontiguousarray(
            (idx.reshape(nblk, P).T < n_c).astype(np.float32)
        )
        mrow = np.ascontiguousarray(
            np.broadcast_to(
                (idx < n_c).astype(ml_dtypes.bfloat16), (P, nblk * P)
            )
        )
        in_maps.append({"bq": bqc, "mcol": mcol, "mrow": mrow})
    global _last_in_maps
    _last_in_maps = in_maps
    res = run_bass_kernel_spmd(nc, in_maps, core_ids=list(range(bs)))
    out = np.array([r["out"][0, 0] for r in res.results], dtype=np.float32)
    return out


# revision 37
# speedup vs baseline: 1.0853x; 1.0629x over previous
"""Trainium2 Bass kernel for nn_DPP: batched masked-Gram logdet minus shared
normalizer logdet.

out[i] = logdet(G * m_i m_i^T + diag(1-m_i)) - logdet(G + I),  G = B^T B

Sharding: data-parallel over the batch dim of x (one sample per NeuronCore,
B replicated). Host-side trick: each core gets B with its sample's SELECTED
columns permuted to the front, so the masked logdet is the logdet of the
LEADING ~n_sel block of the permuted Gram G' (logdet(G+I) is permutation
invariant), and one Gram serves both factorizations.

Device algorithm (per core):
  - G' = Bq^T Bq upper-triangle strips via fp8(e4m3) DoubleRow matmuls
    (fp32 PSUM accum, 2x PE throughput), B loaded in 16 chunked DMAs that
    overlap with the first Gram strips' accumulation chains.
  - Two interleaved left-looking blocked Cholesky factorizations (U-form,
    128-wide panels): A0 = leading-nblk-block masked G' (+ identity pad on
    partial blocks), A1 = G' + I.  A0's panels are OFFSET to pair with A1's
    tail panels.  Panels are software-pipelined: each panel's diagonal
    Schur chain is pre-accumulated (open PSUM group) during the PREVIOUS
    panel's refine, TRSM emits the diag+block1 tiles first so the next
    diagonal closes with a single matmul, and the remaining TRSM tiles /
    trailing accumulations / Gram strips fill the refine latency.
  - Each 128x128 diagonal pivot S is handled matmul-only ("refine" scheme):
      d = diag(S); r = 1/sqrt(d); q = r r^T
      X1 = striu(S) * q; X1T = stril(S) * q      (striu(DSD) = striu(S)*q)
      W = diag(r) (I - X1 + X1@X1)               (approx inv-chol factor)
      F = W^T S W - I                            (small)
      logdet(S) = sum(ln d) + tr F - tr F^2/2 + tr F^3/3
      What = W + W(-F/2 + 3F^2/8)                (What What^T ~ S^{-1})
    Panel: U_strip = What^T @ strip; trailing Schur updates use U (bf16).
    All ln d are batched into one ACT Ln at the end.
"""

import numpy as np
import ml_dtypes

P = 128
N = 2048           # matrix dim (= n columns of B)
NT = N // P        # 16 column tiles
NKT = 16           # contraction tiles (B rows padded 2000 -> 2048)
NKT2 = 8           # fp8 DoubleRow pairs
FT = 512           # free-dim tile for wide matmuls

_CACHE = {}
_last_in_maps = None
_PLAIN_GRAM = False  # CoreSim-only fallback (interp lacks 4D DoubleRow)


def _col_tiles(width_blocks, base_col, diag_first=False):
    """Gram tiling: split cols into <=512 tiles from the strip start."""
    tiles = []
    c = base_col
    end = base_col + width_blocks * P
    if diag_first:
        tiles.append((c, P))
        c += P
    while c < end:
        w = min(FT, end - c)
        tiles.append((c, w))
        c += w
    return tiles


def _panel_tiles(width_blocks, base_col):
    """Panel tiling: [diag P][block1 P][pad to abs 512 grid][512 grid...].
    block1 is split out so the next panel's diagonal Schur term needs only
    the first two TRSM tiles; the rest is 512-grid aligned."""
    end = base_col + width_blocks * P
    tiles = [(base_col, P)]
    c = base_col + P
    if c < end:
        tiles.append((c, P))
        c += P
    if c < end and c % FT:
        w = min(FT - c % FT, end - c)
        tiles.append((c, w))
        c += w
    while c < end:
        w = min(FT, end - c)
        tiles.append((c, w))
        c += w
    return tiles


def _build(nblk):
    import concourse.bass as bass
    import concourse.bacc as bacc
    import concourse.mybir as mybir
    from concourse.bass import ds, ts
    from concourse.masks import (
        make_identity,
        make_upper_triangular,
        make_lower_triangular,
    )
    from concourse.tile import TileContext
    from contextlib import ExitStack

    f32 = mybir.dt.float32
    bf16 = mybir.dt.bfloat16
    f8 = mybir.dt.float8e4
    AF = mybir.ActivationFunctionType
    OP = mybir.AluOpType
    PSUM = bass.MemorySpace.PSUM
    AX = mybir.AxisListType.X
    DR = mybir.MatmulPerfMode.DoubleRow
    OFF = NT - nblk  # m0 panel j runs at step t = j + OFF

    nc = bacc.Bacc()
    # B in block-pair layout [p, kt2, blk, slab, c] flattened per partition:
    # DoubleRow weights slices must be contiguous [P, 2, 128]
    bq = nc.dram_tensor("bq", [P, NKT2 * NT * 2 * P], f8, kind="ExternalInput")
    mcol_d = nc.dram_tensor("mcol", [P, nblk], f32, kind="ExternalInput")
    mrow_d = nc.dram_tensor("mrow", [P, nblk * P], bf16, kind="ExternalInput")
    out_d = nc.dram_tensor("out", [1, 1], f32, kind="ExternalOutput")

    with TileContext(nc) as tc, ExitStack() as stack:
        consts = stack.enter_context(tc.tile_pool(name="consts", bufs=1))
        I128 = consts.tile([P, P], f32, tag="i128")
        make_identity(nc, I128)
        I128b = consts.tile([P, P], bf16, tag="i128b")
        nc.vector.tensor_copy(I128b, I128)
        INEGB = consts.tile([P, P], bf16, tag="inegb")
        nc.vector.tensor_scalar(
            out=INEGB, in0=I128, scalar1=-1.0, scalar2=None, op0=OP.mult
        )
        STRIU = consts.tile([P, P], f32, tag="striu")
        make_upper_triangular(nc, STRIU, val=1.0, diag=False)
        STRIL = consts.tile([P, P], f32, tag="stril")
        make_lower_triangular(nc, STRIL, val=1.0, diag=False)
        STRIUN = consts.tile([P, P], f32, tag="striun")
        make_upper_triangular(nc, STRIUN, val=-1.0, diag=False)
        STRILN = consts.tile([P, P], f32, tag="striln")
        make_lower_triangular(nc, STRILN, val=-1.0, diag=False)
        INEGF = consts.tile([P, P], f32, tag="inegf")
        nc.vector.tensor_scalar(
            out=INEGF, in0=I128, scalar1=-1.0, scalar2=None, op0=OP.mult
        )
        mcol = consts.tile([P, nblk], f32, tag="mcol")
        nc.sync.dma_start(mcol, mcol_d[:, :])
        mrowrep = consts.tile([P, nblk * P], bf16, tag="mrowrep")
        nc.sync.dma_start(mrowrep, mrow_d[:, :])
        acc = consts.tile([P, 2], f32, tag="acc")
        nc.vector.memset(acc, 0.0)
        dstore = consts.tile([P, 2, NT], f32, tag="dstore")
        nc.vector.memset(dstore.rearrange("p a b -> p (a b)"), 1.0)
        onem_all = consts.tile([P, nblk], f32, tag="onem_all")
        nc.vector.tensor_scalar(
            out=onem_all, in0=mcol, scalar1=-1.0, scalar2=1.0,
            op0=OP.mult, op1=OP.add,
        )
        dfix_all = consts.tile([P, nblk, P], f32, tag="dfix_all")
        for i in range(nblk):
            nc.vector.tensor_scalar_mul(dfix_all[:, i, :], I128, onem_all[:, ds(i, 1)])

        gs = []  # gs[i]: [P, (NT-i)*P] bf16, absolute cols i*128..2048
        for i in range(NT):
            gs.append(consts.tile([P, (NT - i) * P], bf16, tag=f"gs{i}", name=f"gs{i}"))
        ub = {}  # panels of the two factorizations (m0: nblk-wide, m1: full)
        for i in range(nblk):
            ub[(0, i)] = consts.tile(
                [P, (nblk - i) * P], bf16, tag=f"ub0_{i}", name=f"ub0_{i}"
            )
        for i in range(NT):
            ub[(1, i)] = consts.tile(
                [P, (NT - i) * P], bf16, tag=f"ub1_{i}", name=f"ub1_{i}"
            )

        bpool = stack.enter_context(tc.tile_pool(name="bpool", bufs=1))
        gpsum = stack.enter_context(tc.tile_pool(name="gram_psum", bufs=1, space=PSUM))
        spool = stack.enter_context(tc.tile_pool(name="strip_pool", bufs=4))
        ppool = stack.enter_context(tc.tile_pool(name="pre_pool", bufs=6))
        rpool = stack.enter_context(tc.tile_pool(name="ref_pool", bufs=4))
        vpool = stack.enter_context(tc.tile_pool(name="vec_pool", bufs=4))
        apsum = stack.enter_context(tc.tile_pool(name="acc_psum", bufs=2, space=PSUM))
        wpsum = stack.enter_context(tc.tile_pool(name="work_psum", bufs=3, space=PSUM))
        dpsum = stack.enter_context(tc.tile_pool(name="diag_psum", bufs=1, space=PSUM))

        bt = bpool.tile([P, NKT2, NT, 2, P], f8, tag="bt")
        btf = bt.rearrange("p k b s c -> p (k b s c)")
        CH = NT * 2 * P  # one kt2 pair-slab chunk
        for kt in range(NKT2):
            nc.sync.dma_start(btf[:, ds(kt * CH, CH)], bq[:, ds(kt * CH, CH)])
        btr = bt.rearrange("p k b s c -> p k s b c")

        # round-robin engine pickers for balanced elementwise work
        _tt_state = 0
        _cp_state = 0

        def tt_eng():
            nonlocal _tt_state
            _tt_state += 1
            return (nc.vector, nc.gpsimd)[_tt_state % 2]

        def copy_rr(out, in_):
            nonlocal _cp_state
            _cp_state += 1
            if _cp_state % 2 == 0:
                nc.scalar.copy(out, in_)
            else:
                nc.vector.tensor_copy(out, in_)

        def gram_mm_chain(pt, i, c0, w, kt):
            if _PLAIN_GRAM:
                for s in range(2):
                    nc.tensor.matmul(
                        pt[:, :w],
                        bt[:, kt, i, s, :],
                        btr[:, kt, s, ds(c0 // P, w // P), :],
                        start=(kt == 0 and s == 0),
                        stop=(kt == NKT2 - 1 and s == 1),
                    )
                return
            nc.tensor.matmul(
                pt[:, :w],
                bt[:, kt, i, :, :],
                btr[:, kt, :, ds(c0 // P, w // P), :],
                start=(kt == 0),
                stop=(kt == NKT2 - 1),
                perf_mode=DR,
            )

        def gram_warmup():
            """Strips 0 and 1, kt-major across 6 concurrent PSUM chains so the
            Gram accumulation pipelines with the 16 chunked B DMAs."""
            chains = []
            pools = [(gpsum, "gp"), (apsum, "ap"), (apsum, "ap"),
                     (wpsum, "w"), (wpsum, "w"), (wpsum, "w")]
            tiles01 = [(0, c0, w) for (c0, w) in _col_tiles(NT, 0)] + [
                (1, c0, w) for (c0, w) in _col_tiles(NT - 1, P)
            ]
            for (i, c0, w), (pool, tg) in zip(tiles01[:6], pools):
                chains.append((i, c0, w, pool.tile([P, FT], f32, tag=tg, name="gw")))
            for kt in range(NKT2):
                for (i, c0, w, pt) in chains:
                    gram_mm_chain(pt, i, c0, w, kt)
            for (i, c0, w, pt) in chains:
                nc.scalar.copy(gs[i][:, ds(c0 - i * P, w)], pt[:, :w])
                gram_done[(i, c0)] = True
            for (i, c0, w) in tiles01[6:]:
                pt = gpsum.tile([P, FT], f32, tag="gp", name="pt")
                for kt in range(NKT2):
                    gram_mm_chain(pt, i, c0, w, kt)
                nc.scalar.copy(gs[i][:, ds(c0 - i * P, w)], pt[:, :w])
                gram_done[(i, c0)] = True

        _neg_state = [0]

        # ---------- software-pipeline state ----------
        from collections import deque

        gram_done = {}     # (strip, c0) -> emitted
        gramq = deque()
        tailq = {0: deque(), 1: deque()}
        emitted = {}       # (m, panel, tix) -> TRSM tile emitted
        pre_state = {0: None, 1: None}
        blk1_state = {0: None, 1: None}
        nblks_of = {0: nblk, 1: NT}
        _ptiles = {}
        for _i in range(nblk):
            _ptiles[(0, _i)] = _panel_tiles(nblk - _i, _i * P)
        for _i in range(NT):
            _ptiles[(1, _i)] = _panel_tiles(NT - _i, _i * P)

        def _gtile_of(i, col):
            return i * P + ((col - i * P) // FT) * FT

        def tile_ix_of(m, j, col):
            for tix, (c0, w) in enumerate(_ptiles[(m, j)]):
                if c0 <= col < c0 + w:
                    return tix
            raise AssertionError((m, j, col))

        def gram_gen(i):
            for (c0, w) in _col_tiles(NT - i, i * P):
                pt = gpsum.tile([P, FT], f32, tag="gp", name="pt")
                for kt in range(NKT2):
                    gram_mm_chain(pt, i, c0, w, kt)
                    if kt % 2 == 1 and kt < NKT2 - 1:
                        yield True
                nc.scalar.copy(gs[i][:, ds(c0 - i * P, w)], pt[:, :w])
                gram_done[(i, c0)] = True
                yield True

        def gram_master():
            if not gramq:
                return False
            g = gramq[0]
            try:
                return next(g)
            except StopIteration:
                gramq.popleft()
                return True

        def pull_gram_until(i, c0):
            guard = 0
            while (i, c0) not in gram_done:
                assert gramq, ("gram starved", i, c0)
                gram_master()
                guard += 1
                assert guard < 100000

        def tail_master(m):
            if not tailq[m]:
                return False
            g = tailq[m][0]
            try:
                return next(g)
            except StopIteration:
                tailq[m].popleft()
                return True

        def pull_tail_until(m, key):
            guard = 0
            while key not in emitted:
                assert tailq[m], ("tail starved", m, key)
                tail_master(m)
                guard += 1
                assert guard < 100000

        def diag_pre_vec(m, i):
            """Mask/I prep of gs diag block for panel i (no Schur terms)."""
            gsl = gs[i][:, ds(0, P)]
            pre = ppool.tile([P, P], bf16, tag=f"pre{m}", name="pre")
            if m == 1:
                nc.vector.tensor_add(pre, gsl, I128)
            else:
                tmp = ppool.tile([P, P], f32, tag="mtmp", name="mtmp")
                nc.gpsimd.tensor_mul(tmp, gsl, mrowrep[:, ds(i * P, P)])
                nc.gpsimd.tensor_scalar_mul(tmp, tmp, mcol[:, ds(i, 1)])
                nc.gpsimd.tensor_add(pre, tmp, dfix_all[:, i, :])
            return pre

        def diag_ap_mm(apt, m, n, j, start, stop=False):
            nc.tensor.matmul(
                apt,
                ub[(m, j)][:, ds((n - j) * P, P)],
                ub[(m, j)][:, ds((n - j) * P, P)],
                start=start,
                stop=stop,
            )

        def start_pre(m, n):
            if m == 0:
                return  # m0 chains are short; emitted force-closed at close
            pre_state[m] = {"n": n, "j": 0, "apt": None, "pre": None,
                            "started": False}

        def advance_pre(m, budget=6, force=False):
            st = pre_state[m]
            if st is None:
                return None
            n = st["n"]
            if st["pre"] is None:
                gc = _gtile_of(n, n * P)
                if (n, gc) not in gram_done:
                    if force:
                        pull_gram_until(n, gc)
                    else:
                        return False
                st["pre"] = diag_pre_vec(m, n)
                st["apt"] = dpsum.tile([P, P], f32, tag="dp", name="dp")
                nc.tensor.matmul(st["apt"], INEGB, st["pre"],
                                 start=True, stop=False)
                st["started"] = True
                budget -= 1
            while st["j"] <= n - 2:
                j = st["j"]
                key = (m, j, tile_ix_of(m, j, n * P))
                if key not in emitted:
                    if force:
                        pull_tail_until(m, key)
                    else:
                        return False
                diag_ap_mm(st["apt"], m, n, j, not st["started"])
                st["started"] = True
                st["j"] += 1
                budget -= 1
                if budget <= 0 and not force:
                    return True
            return None  # ready for close

        def close_diag(m, i, cx):
            sblk, sb = cx["sblk"], cx["sb"]
            if i == 0:
                gsl = gs[0][:, ds(0, P)]
                if m == 1:
                    nc.vector.tensor_add(sblk, gsl, I128)
                else:
                    tmp = ppool.tile([P, P], f32, tag="mtmp", name="mtmp")
                    nc.vector.tensor_mul(tmp, gsl, mrowrep[:, ds(0, P)])
                    nc.vector.tensor_scalar_mul(tmp, tmp, mcol[:, ds(0, 1)])
                    nc.vector.tensor_add(sblk, tmp, dfix_all[:, 0, :])
            elif m == 1:
                advance_pre(m, force=True)
                st = pre_state[m]
                assert st is not None and st["n"] == i
                diag_ap_mm(st["apt"], m, i, i - 1, not st["started"], stop=True)
                cx["sneg"] = st["apt"]
                nc.scalar.mul(sb, st["apt"], -1.0)
                pre_state[m] = None
                return
            else:
                gc = _gtile_of(i, i * P)
                pull_gram_until(i, gc)
                for j in range(i - 1):
                    pull_tail_until(0, (0, j, tile_ix_of(0, j, i * P)))
                pre = diag_pre_vec(0, i)
                apw = wpsum.tile([P, FT], f32, tag="w", name="apw")
                nc.tensor.matmul(apw[:, :P], INEGB, pre, start=True, stop=False)
                for j in range(i):
                    diag_ap_mm(apw[:, :P], 0, i, j, False, stop=(j == i - 1))
                cx["sneg"] = apw[:, :P]
                nc.scalar.mul(sb, apw[:, :P], -1.0)
                return
            (nc.scalar.copy if m == 1 else nc.gpsimd.tensor_copy)(sb, sblk)

        def new_panel(m, i):
            wblk = nblks_of[m] - i
            return {
                "i": i,
                "tiles": _ptiles[(m, i)],
                "strip": spool.tile([P, wblk * P], bf16, tag="strip", name="strip"),
                "sblk": rpool.tile([P, P], f32, tag="sblk", name="sblk"),
                "sb": rpool.tile([P, P], bf16, tag="sb", name="sb"),
                "sneg": None,
            }

        def start_blk1(m, i, cx):
            blk1_state[m] = None
            if len(cx["tiles"]) < 2:
                return
            if i == 0:
                if m == 0:
                    nc.vector.tensor_mul(
                        cx["strip"][:, ds(P, P)], gs[0][:, ds(P, P)],
                        mrowrep[:, ds(P, P)],
                    )
                return
            if m == 0:
                blk1_state[m] = {"i": i, "cx": cx, "forced": True}
                return
            blk1_state[m] = {"i": i, "cx": cx, "j": 0, "apt": None,
                             "pre1": None, "prepped": False, "started": False,
                             "forced": False}

        def advance_blk1(m, budget=6, force=False):
            st = blk1_state[m]
            if st is None or st.get("forced"):
                return None
            i = st["i"]
            cx = st["cx"]
            c0, _w = cx["tiles"][1]
            if not st["prepped"]:
                gc = _gtile_of(i, c0)
                if (i, gc) not in gram_done:
                    if force:
                        pull_gram_until(i, gc)
                    else:
                        return False
                st["apt"] = dpsum.tile([P, P], f32, tag="dp1", name="dp1")
                nc.tensor.matmul(
                    st["apt"], INEGB, gs[i][:, ds(c0 - i * P, P)],
                    start=True, stop=False,
                )
                st["started"] = True
                st["prepped"] = True
                budget -= 1
            while st["j"] <= i - 2:
                j = st["j"]
                k1 = (m, j, tile_ix_of(m, j, i * P))
                k2 = (m, j, tile_ix_of(m, j, c0))
                if k1 not in emitted or k2 not in emitted:
                    if force:
                        pull_tail_until(m, k1)
                        pull_tail_until(m, k2)
                    else:
                        return False
                nc.tensor.matmul(
                    st["apt"],
                    ub[(m, j)][:, ds((i - j) * P, P)],
                    ub[(m, j)][:, ds(c0 - j * P, P)],
                    start=not st["started"],
                    stop=False,
                )
                st["started"] = True
                st["j"] += 1
                budget -= 1
                if budget <= 0 and not force:
                    return True
            return None  # ready for blk1_finish

        def blk1_finish(m):
            st = blk1_state[m]
            if st is None:
                return
            i = st["i"]
            cx = st["cx"]
            c0, _w = cx["tiles"][1]
            dst = cx["strip"][:, ds(c0 - i * P, P)]
            if st.get("forced"):
                gc = _gtile_of(i, c0)
                pull_gram_until(i, gc)
                for j in range(i):
                    if j < i - 1:
                        pull_tail_until(m, (m, j, tile_ix_of(m, j, i * P)))
                    pull_tail_until(m, (m, j, tile_ix_of(m, j, c0)))
                p1 = ppool.tile([P, P], bf16, tag="p1", name="p1")
                nc.gpsimd.tensor_mul(
                    p1, gs[i][:, ds(c0 - i * P, P)], mrowrep[:, ds(c0, P)]
                )
                apw = wpsum.tile([P, FT], f32, tag="w", name="apw")
                nc.tensor.matmul(apw[:, :P], INEGB, p1, start=True, stop=False)
                for j in range(i):
                    nc.tensor.matmul(
                        apw[:, :P],
                        ub[(m, j)][:, ds((i - j) * P, P)],
                        ub[(m, j)][:, ds(c0 - j * P, P)],
                        start=False,
                        stop=(j == i - 1),
                    )
                nc.scalar.mul(dst, apw[:, :P], -1.0)
                blk1_state[m] = None
                return
            advance_blk1(m, force=True)
            key = (m, i - 1, tile_ix_of(m, i - 1, c0))
            pull_tail_until(m, key)
            nc.tensor.matmul(
                st["apt"],
                ub[(m, i - 1)][:, ds(P, P)],
                ub[(m, i - 1)][:, ds(c0 - (i - 1) * P, P)],
                start=not st["started"],
                stop=True,
            )
            nc.scalar.mul(dst, st["apt"], -1.0)
            blk1_state[m] = None

        def refine_gen(m, i, cx):
            """Pivot-block factor; yields at cross-engine handoffs so filler
            matmuls can be emitted between dependent steps.  m1's elementwise
            ops ride DVE, m0's ride Pool (reduces are DVE-only)."""
            E = nc.vector if m == 1 else nc.gpsimd
            sb = cx["sb"]
            sneg = cx["sneg"]
            ssrc = sneg if sneg is not None else cx["sblk"]
            sgn = -1.0 if sneg is not None else 1.0
            dcol = dstore[:, m, ds(i, 1)]
            scr0 = rpool.tile([P, P], f32, tag="scr0", name="scr0")
            nc.vector.tensor_mul(scr0, ssrc, INEGF if sneg is not None else I128)
            nc.vector.tensor_reduce(dcol, scr0, AX, OP.add)
            rinv = vpool.tile([P, 1], f32, tag="rinv", name="rinv")
            nc.vector.reciprocal(rinv, dcol)
            yt = rpool.tile([P, P], f32, tag="yt", name="yt")
            nc.vector.tensor_mul(yt, ssrc, STRILN if sneg is not None else STRIL)
            yu = rpool.tile([P, P], f32, tag="yu", name="yu")
            nc.vector.tensor_mul(yu, ssrc, STRIUN if sneg is not None else STRIU)
            yield
            rt_ps = wpsum.tile([P, FT], f32, tag="w", name="rt_ps")
            nc.tensor.transpose(rt_ps[:1, :P], rinv, I128)
            rcol = vpool.tile([P, 1], f32, tag="rcol", name="rcol")
            nc.scalar.sqrt(rcol, rinv)
            rrow = vpool.tile([1, P], bf16, tag="rrow", name="rrow")
            nc.scalar.sqrt(rrow, rt_ps[:1, :P])
            yield
            q_ps = wpsum.tile([P, FT], f32, tag="w", name="q_ps")
            nc.tensor.matmul(q_ps[:, :P], rrow, rrow, start=True, stop=True)
            x1 = rpool.tile([P, P], bf16, tag="x1", name="x1")
            nc.vector.tensor_mul(x1, yu, q_ps[:, :P])
            x1t = rpool.tile([P, P], bf16, tag="x1t", name="x1t")
            nc.vector.tensor_mul(x1t, yt, q_ps[:, :P])
            yield
            # x2_ps accumulates X1@X1 - X1 + I entirely on PE
            x2_ps = wpsum.tile([P, FT], f32, tag="w", name="x2_ps")
            nc.tensor.matmul(x2_ps[:, :P], x1t, x1, start=True, stop=False)
            nc.tensor.matmul(x2_ps[:, :P], INEGB, x1, start=False, stop=False)
            nc.tensor.matmul(x2_ps[:, :P], I128b, I128b, start=False, stop=True)
            wfac = rpool.tile([P, P], bf16, tag="wfac", name="wfac")
            if m == 1:
                nc.vector.tensor_scalar_mul(wfac, x2_ps[:, :P], rcol)
            else:
                nc.scalar.activation(
                    wfac, x2_ps[:, :P], AF.Copy, scale=rcol)
            yield
            last = len(cx["tiles"]) == 1
            wt = None
            if not last:
                wt_ps = wpsum.tile([P, FT * 2], bf16, tag="w", name="wt_ps")
                nc.tensor.transpose(wt_ps[:, :P], wfac, I128b)
                wt = rpool.tile([P, P], bf16, tag="wt", name="wt")
                nc.scalar.copy(wt, wt_ps[:, :P])
            sw_ps = wpsum.tile([P, FT], f32, tag="w", name="sw_ps")
            nc.tensor.matmul(sw_ps[:, :P], sb, wfac, start=True, stop=True)
            swt = rpool.tile([P, P], bf16, tag="swt", name="swt")
            nc.scalar.copy(swt, sw_ps[:, :P])
            yield
            # fpi_ps accumulates W^T S W - I on PE: result is F itself
            fpi_ps = wpsum.tile([P, FT], f32, tag="w", name="fpi_ps")
            nc.tensor.matmul(fpi_ps[:, :P], wfac, swt, start=True, stop=False)
            nc.tensor.matmul(fpi_ps[:, :P], INEGB, I128b, start=False, stop=True)
            ff = rpool.tile([P, P], bf16, tag="ff", name="ff")
            nc.scalar.copy(ff, fpi_ps[:, :P])
            if not last:
                fs = rpool.tile([P, P], bf16, tag="fs", name="fs")
                E.tensor_scalar_mul(fs, ff, -0.5)
            yield
            if not last:
                wh_ps = wpsum.tile([P, FT], f32, tag="w", name="wh_ps")
                nc.tensor.matmul(wh_ps[:, :P], wt, fs, start=True, stop=True)
                what = rpool.tile([P, P], bf16, tag="what", name="what")
                nc.vector.tensor_add(what, wh_ps[:, :P], wfac)
                cx["what"] = what
                if m == 0:
                    whatm = rpool.tile([P, P], bf16, tag="whatm", name="whatm")
                    nc.gpsimd.tensor_scalar_mul(whatm, what, mcol[:, ds(i, 1)])
                    cx["whatm"] = whatm
            # logdet trace series, from bf16 copies, off the What chain
            trf = vpool.tile([P, 1], f32, tag="trf", name="trf")
            scr1 = rpool.tile([P, P], f32, tag="scr1", name="scr1")
            nc.gpsimd.tensor_mul(scr1, ff, I128)
            nc.vector.tensor_reduce(trf, scr1, AX, OP.add)
            trf2 = vpool.tile([P, 1], f32, tag="trf2", name="trf2")
            scr2 = rpool.tile([P, P], f32, tag="scr2", name="scr2")
            nc.gpsimd.tensor_mul(scr2, ff, ff)
            nc.vector.tensor_reduce(trf2, scr2, AX, OP.add)
            t1 = vpool.tile([P, 1], f32, tag="t1", name="t1")
            nc.vector.tensor_scalar(
                out=t1, in0=trf2, scalar1=-0.5, scalar2=None, op0=OP.mult
            )
            nc.vector.tensor_add(t1, t1, trf)
            nc.vector.tensor_add(acc[:, ds(m, 1)], acc[:, ds(m, 1)], t1)

        def trsm_tile(m, i, cx, tix):
            c0, w = cx["tiles"][tix]
            if m == 1 and i == 0 and tix > 0:
                rhs = gs[0][:, ds(c0, w)]
            elif tix == 0:
                rhs = cx["sb"]
            else:
                rhs = cx["strip"][:, ds(c0 - i * P, w)]
            lhs = cx["what"] if tix == 0 else cx.get("whatm", cx["what"])
            tp = wpsum.tile([P, FT], f32, tag="w", name="tp")
            nc.tensor.matmul(tp[:, :w], lhs, rhs, start=True, stop=True)
            dst = ub[(m, i)][:, ds(c0 - i * P, w)]
            if tix <= 1:
                nc.vector.tensor_copy(dst, tp[:, :w])
            else:
                copy_rr(dst, tp[:, :w])

        def trsm_head(m, i, cx):
            if len(cx["tiles"]) > 1:
                trsm_tile(m, i, cx, 1)
                emitted[(m, i, 1)] = True

        def tail_gen(m, i, cx):
            tiles = cx["tiles"]
            strip = cx["strip"]
            for tix in range(2, len(tiles)):
                c0, w = tiles[tix]
                if not (m == 1 and i == 0):
                    for col in (c0, c0 + w - 1):
                        gc = _gtile_of(i, col)
                        while (i, gc) not in gram_done:
                            yield False
                    gsl = gs[i][:, ds(c0 - i * P, w)]
                    dst = strip[:, ds(c0 - i * P, w)]
                    if i > 0:
                        if m == 0:
                            tmpm = spool.tile([P, FT], bf16, tag="ptmp",
                                              name="tmpm")
                            tt_eng().tensor_mul(
                                tmpm[:, :w], gsl, mrowrep[:, ds(c0, w)]
                            )
                            neg_src = tmpm[:, :w]
                        else:
                            neg_src = gsl
                        ap = apsum.tile([P, FT], f32, tag="ap", name="ap")
                        nc.tensor.matmul(
                            ap[:, :w], INEGB, neg_src, start=True, stop=False
                        )
                        for j in range(i):
                            nc.tensor.matmul(
                                ap[:, :w],
                                ub[(m, j)][:, ds((i - j) * P, P)],
                                ub[(m, j)][:, ds(c0 - j * P, w)],
                                start=False,
                                stop=(j == i - 1),
                            )
                            if j % 2 == 1 and j < i - 1:
                                yield True
                        nonlocal_ns = _neg_state[0] = _neg_state[0] + 1
                        if nonlocal_ns % 3 == 0:
                            nc.scalar.mul(dst, ap[:, :w], -1.0)
                        else:
                            nc.vector.tensor_scalar(
                                out=dst, in0=ap[:, :w], scalar1=-1.0,
                                scalar2=None, op0=OP.mult,
                            )
                    else:
                        tt_eng().tensor_mul(dst, gsl, mrowrep[:, ds(c0, w)])
                    yield True
                trsm_tile(m, i, cx, tix)
                emitted[(m, i, tix)] = True
                yield True

        # ---- software-pipelined emission --------------------------------
        gram_warmup()

        for t in range(NT):
            panels = [(1, t)]
            j0 = t - OFF
            if 0 <= j0 < nblk:
                panels.append((0, j0))
            cxs = {}
            gens = []
            live = []
            # m1 closes and primes its refine first; m0's boundary work then
            # lands in m1's early refine window (PE idle during sqrt/ttr).
            for (m, i) in panels:
                cxs[m] = new_panel(m, i)
                close_diag(m, i, cxs[m])
                start_blk1(m, i, cxs[m])
                g = refine_gen(m, i, cxs[m])
                next(g)
                gens.append(g)
                live.append(g)
            for (m, i) in panels:
                if i + 1 < nblks_of[m]:
                    start_pre(m, i + 1)
            if t + 2 < NT:
                gramq.append(gram_gen(t + 2))

            def mk(fn, *a):
                return lambda: fn(*a)

            tasks = [mk(tail_master, 1), mk(tail_master, 0), mk(gram_master)]
            for (m, i) in panels:
                if blk1_state[m] is not None:
                    tasks.append(mk(advance_blk1, m))
                if pre_state[m] is not None:
                    tasks.append(mk(advance_pre, m))
            fi = 0
            while live:
                for g in list(live):
                    try:
                        next(g)
                    except StopIteration:
                        live.remove(g)
                prog = 0
                attempts = 0
                while tasks and prog < 3 and attempts < 2 * len(tasks):
                    tk = tasks[fi % len(tasks)]
                    fi += 1
                    attempts += 1
                    r = tk()
                    if r is None:
                        tasks.remove(tk)
                    elif r:
                        prog += 1
            for (m, i) in panels:
                blk1_finish(m)
                trsm_head(m, i, cxs[m])
                if len(cxs[m]["tiles"]) > 2:
                    tailq[m].append(tail_gen(m, i, cxs[m]))

        guard = 0
        while tailq[0] or tailq[1] or gramq:
            p1 = tail_master(1)
            p0 = tail_master(0)
            pg = gram_master()
            guard = 0 if (p1 or p0 or pg) else guard + 1
            assert guard < 1000, "drain deadlock"

        # -------- final: batched Ln(d), partition-sum via matmul ------
        lnall = vpool.tile([P, 2, NT], f32, tag="lnall", name="lnall")
        nc.scalar.activation(
            lnall.rearrange("p a b -> p (a b)"),
            dstore.rearrange("p a b -> p (a b)"), AF.Ln,
        )
        ln0 = vpool.tile([P, 1], f32, tag="ln0", name="ln0")
        nc.vector.tensor_reduce(ln0, lnall[:, 0, :], AX, OP.add)
        ln1 = vpool.tile([P, 1], f32, tag="ln1", name="ln1")
        nc.vector.tensor_reduce(ln1, lnall[:, 1, :], AX, OP.add)
        accd = vpool.tile([P, 1], f32, tag="accd", name="accd")
        nc.vector.tensor_sub(accd, acc[:, 0:1], acc[:, 1:2])
        nc.vector.tensor_add(accd, accd, ln0)
        nc.vector.tensor_sub(accd, accd, ln1)
        ones = vpool.tile([P, 1], f32, tag="ones", name="ones")
        nc.vector.memset(ones, 1.0)
        r_ps = wpsum.tile([P, FT], f32, tag="w", name="r_ps")
        nc.tensor.matmul(r_ps[:1, :1], accd, ones, start=True, stop=True)
        res = vpool.tile([1, 1], f32, tag="res", name="res")
        nc.vector.tensor_copy(res, r_ps[:1, :1])
        nc.sync.dma_start(out_d[:, :], res)

    nc.finalize()
    return nc


def kernel(x, B):
    """Full inputs -> full output. x: [8, 2048] int32, B: [2000, 2048] f32."""
    from concourse.bass_utils import run_bass_kernel_spmd

    bs, n = x.shape
    k = B.shape[0]
    assert n == N and bs == 8

    B8 = B.astype(ml_dtypes.float8_e4m3fn)
    percore = []
    nblk = 1
    for c in range(bs):
        m = x[c] == 1
        sel = np.where(m)[0]
        unsel = np.where(~m)[0]
        n_c = len(sel)
        percore.append((np.concatenate([sel, unsel]), n_c))
        nblk = max(nblk, -(-n_c // P))

    if nblk not in _CACHE:
        _CACHE[nblk] = _build(nblk)
    nc = _CACHE[nblk]

    in_maps = []
    for perm, n_c in percore:
        bqc = np.zeros((N, N), dtype=ml_dtypes.float8_e4m3fn)
        bqc[:k, :] = B8[:, perm]
        # [p, kt2, blk, slab, c] layout, flattened per partition row
        bqc = np.ascontiguousarray(
            bqc.reshape(NKT2, 2, P, NT, P)
            .transpose(2, 0, 3, 1, 4)
            .reshape(P, NKT2 * NT * 2 * P)
        )
        idx = np.arange(nblk * P)
        mcol = np.ascontiguousarray(
            (idx.reshape(nblk, P).T < n_c).astype(np.float32)
        )
        mrow = np.ascontiguousarray(
            np.broadcast_to(
                (idx < n_c).astype(ml_dtypes.bfloat16), (P, nblk * P)
            )
        )
        in_maps.append({"bq": bqc, "mcol": mcol, "mrow": mrow})
    global _last_in_maps
    _last_in_maps = in_maps
    res = run_bass_kernel_spmd(nc, in_maps, core_ids=list(range(bs)))
    out = np.array([r["out"][0, 0] for r in res.results], dtype=np.float32)
    return out


# revision 42
# speedup vs baseline: 1.1203x; 1.0323x over previous
"""Trainium2 Bass kernel for nn_DPP: batched masked-Gram logdet minus shared
normalizer logdet.

out[i] = logdet(G * m_i m_i^T + diag(1-m_i)) - logdet(G + I),  G = B^T B

Sharding: data-parallel over the batch dim of x (one sample per NeuronCore,
B replicated). Host-side trick: each core gets B with its sample's SELECTED
columns permuted to the front, so the masked logdet is the logdet of the
LEADING ~n_sel block of the permuted Gram G' (logdet(G+I) is permutation
invariant), and one Gram serves both factorizations.

Device algorithm (per core):
  - G' = Bq^T Bq upper-triangle strips via fp8(e4m3) DoubleRow matmuls
    (fp32 PSUM accum, 2x PE throughput), B loaded in 16 chunked DMAs that
    overlap with the first Gram strips' accumulation chains.
  - Two interleaved left-looking blocked Cholesky factorizations (U-form,
    128-wide panels): A0 = leading-nblk-block masked G' (+ identity pad on
    partial blocks), A1 = G' + I.  A0's panels are OFFSET to pair with A1's
    tail panels.  Panels are software-pipelined: each panel's diagonal
    Schur chain is pre-accumulated (open PSUM group) during the PREVIOUS
    panel's refine, TRSM emits the diag+block1 tiles first so the next
    diagonal closes with a single matmul, and the remaining TRSM tiles /
    trailing accumulations / Gram strips fill the refine latency.
  - Each 128x128 diagonal pivot S is handled matmul-only ("refine" scheme):
      d = diag(S); r = 1/sqrt(d); q = r r^T
      X1 = striu(S) * q; X1T = stril(S) * q      (striu(DSD) = striu(S)*q)
      W = diag(r) (I - X1 + X1@X1)               (approx inv-chol factor)
      F = W^T S W - I                            (small)
      logdet(S) = sum(ln d) + tr F - tr F^2/2 + tr F^3/3
      What = W + W(-F/2 + 3F^2/8)                (What What^T ~ S^{-1})
    Panel: U_strip = What^T @ strip; trailing Schur updates use U (bf16).
    All ln d are batched into one ACT Ln at the end.
"""

import numpy as np
import ml_dtypes

P = 128
N = 2048           # matrix dim (= n columns of B)
NT = N // P        # 16 column tiles
NKT = 16           # contraction tiles (B rows padded 2000 -> 2048)
NKT2 = 8           # fp8 DoubleRow pairs
FT = 512           # free-dim tile for wide matmuls

_CACHE = {}
_last_in_maps = None
_PLAIN_GRAM = False  # CoreSim-only fallback (interp lacks 4D DoubleRow)


def _col_tiles(width_blocks, base_col, diag_first=False):
    """Gram tiling: split cols into <=512 tiles from the strip start."""
    tiles = []
    c = base_col
    end = base_col + width_blocks * P
    if diag_first:
        tiles.append((c, P))
        c += P
    while c < end:
        w = min(FT, end - c)
        tiles.append((c, w))
        c += w
    return tiles


def _panel_tiles(width_blocks, base_col):
    """Panel tiling: [diag P][block1 P][pad to abs 512 grid][512 grid...].
    block1 is split out so the next panel's diagonal Schur term needs only
    the first two TRSM tiles; the rest is 512-grid aligned."""
    end = base_col + width_blocks * P
    tiles = [(base_col, P)]
    c = base_col + P
    if c < end:
        tiles.append((c, P))
        c += P
    if c < end and c % FT:
        w = min(FT - c % FT, end - c)
        tiles.append((c, w))
        c += w
    while c < end:
        w = min(FT, end - c)
        tiles.append((c, w))
        c += w
    return tiles


def _build(nblk):
    import concourse.bass as bass
    import concourse.bacc as bacc
    import concourse.mybir as mybir
    from concourse.bass import ds, ts
    from concourse.masks import (
        make_identity,
        make_upper_triangular,
        make_lower_triangular,
    )
    from concourse.tile import TileContext
    from contextlib import ExitStack

    f32 = mybir.dt.float32
    bf16 = mybir.dt.bfloat16
    f8 = mybir.dt.float8e4
    AF = mybir.ActivationFunctionType
    OP = mybir.AluOpType
    PSUM = bass.MemorySpace.PSUM
    AX = mybir.AxisListType.X
    DR = mybir.MatmulPerfMode.DoubleRow
    OFF = NT - nblk  # m0 panel j runs at step t = j + OFF

    nc = bacc.Bacc()
    # B in block-pair layout [p, kt2, blk, slab, c] flattened per partition:
    # DoubleRow weights slices must be contiguous [P, 2, 128]
    bq = nc.dram_tensor("bq", [P, NKT2 * NT * 2 * P], f8, kind="ExternalInput")
    mcol_d = nc.dram_tensor("mcol", [P, nblk], f32, kind="ExternalInput")
    mrow_d = nc.dram_tensor("mrow", [P, nblk * P], bf16, kind="ExternalInput")
    out_d = nc.dram_tensor("out", [1, 1], f32, kind="ExternalOutput")

    with TileContext(nc) as tc, ExitStack() as stack:
        consts = stack.enter_context(tc.tile_pool(name="consts", bufs=1))
        I128 = consts.tile([P, P], f32, tag="i128")
        make_identity(nc, I128)
        I128b = consts.tile([P, P], bf16, tag="i128b")
        nc.vector.tensor_copy(I128b, I128)
        INEGB = consts.tile([P, P], bf16, tag="inegb")
        nc.vector.tensor_scalar(
            out=INEGB, in0=I128, scalar1=-1.0, scalar2=None, op0=OP.mult
        )
        STRIU = consts.tile([P, P], f32, tag="striu")
        make_upper_triangular(nc, STRIU, val=1.0, diag=False)
        STRIL = consts.tile([P, P], f32, tag="stril")
        make_lower_triangular(nc, STRIL, val=1.0, diag=False)
        STRIUN = consts.tile([P, P], f32, tag="striun")
        make_upper_triangular(nc, STRIUN, val=-1.0, diag=False)
        STRILN = consts.tile([P, P], f32, tag="striln")
        make_lower_triangular(nc, STRILN, val=-1.0, diag=False)
        INEGF = consts.tile([P, P], f32, tag="inegf")
        nc.vector.tensor_scalar(
            out=INEGF, in0=I128, scalar1=-1.0, scalar2=None, op0=OP.mult
        )
        mcol = consts.tile([P, nblk], f32, tag="mcol")
        nc.sync.dma_start(mcol, mcol_d[:, :])
        mrowrep = consts.tile([P, nblk * P], bf16, tag="mrowrep")
        nc.sync.dma_start(mrowrep, mrow_d[:, :])
        acc = consts.tile([P, 2], f32, tag="acc")
        nc.vector.memset(acc, 0.0)
        dstore = consts.tile([P, 2, NT], f32, tag="dstore")
        nc.vector.memset(dstore.rearrange("p a b -> p (a b)"), 1.0)
        onem_all = consts.tile([P, nblk], f32, tag="onem_all")
        nc.vector.tensor_scalar(
            out=onem_all, in0=mcol, scalar1=-1.0, scalar2=1.0,
            op0=OP.mult, op1=OP.add,
        )
        dfix_all = consts.tile([P, nblk, P], f32, tag="dfix_all")
        for i in range(nblk):
            nc.vector.tensor_scalar_mul(dfix_all[:, i, :], I128, onem_all[:, ds(i, 1)])

        gs = []  # gs[i]: [P, (NT-i)*P] bf16, absolute cols i*128..2048
        for i in range(NT):
            gs.append(consts.tile([P, (NT - i) * P], bf16, tag=f"gs{i}", name=f"gs{i}"))
        ub = {}  # panels of the two factorizations (m0: nblk-wide, m1: full)
        for i in range(nblk):
            ub[(0, i)] = consts.tile(
                [P, (nblk - i) * P], bf16, tag=f"ub0_{i}", name=f"ub0_{i}"
            )
        for i in range(NT):
            ub[(1, i)] = consts.tile(
                [P, (NT - i) * P], bf16, tag=f"ub1_{i}", name=f"ub1_{i}"
            )

        bpool = stack.enter_context(tc.tile_pool(name="bpool", bufs=1))
        gpsum = stack.enter_context(tc.tile_pool(name="gram_psum", bufs=1, space=PSUM))
        spool = stack.enter_context(tc.tile_pool(name="strip_pool", bufs=4))
        ppool = stack.enter_context(tc.tile_pool(name="pre_pool", bufs=6))
        rpool = stack.enter_context(tc.tile_pool(name="ref_pool", bufs=4))
        vpool = stack.enter_context(tc.tile_pool(name="vec_pool", bufs=4))
        apsum = stack.enter_context(tc.tile_pool(name="acc_psum", bufs=2, space=PSUM))
        wpsum = stack.enter_context(tc.tile_pool(name="work_psum", bufs=3, space=PSUM))
        dpsum = stack.enter_context(tc.tile_pool(name="diag_psum", bufs=1, space=PSUM))

        bt = bpool.tile([P, NKT2, NT, 2, P], f8, tag="bt")
        btf = bt.rearrange("p k b s c -> p (k b s c)")
        CH = NT * 2 * P   # one kt2 pair-slab chunk
        CQ = 8 * 2 * P    # first 8 column blocks of a chunk
        # first-half chunks first: strips 0/1's early tiles (blocks 0-7) all
        # live there, so panels 0-1 start ~8us earlier
        for kt in range(NKT2):
            nc.sync.dma_start(btf[:, ds(kt * CH, CQ)], bq[:, ds(kt * CH, CQ)])
        for kt in range(NKT2):
            nc.sync.dma_start(
                btf[:, ds(kt * CH + CQ, CH - CQ)], bq[:, ds(kt * CH + CQ, CH - CQ)]
            )
        btr = bt.rearrange("p k b s c -> p k s b c")

        # round-robin engine pickers for balanced elementwise work
        _tt_state = 0
        _cp_state = 0

        def tt_eng():
            nonlocal _tt_state
            _tt_state += 1
            return (nc.vector, nc.gpsimd)[_tt_state % 2]

        def copy_rr(out, in_):
            nonlocal _cp_state
            _cp_state += 1
            if _cp_state % 2 == 0:
                nc.scalar.copy(out, in_)
            else:
                nc.vector.tensor_copy(out, in_)

        def gram_mm_chain(pt, i, c0, w, kt):
            if _PLAIN_GRAM:
                for s in range(2):
                    nc.tensor.matmul(
                        pt[:, :w],
                        bt[:, kt, i, s, :],
                        btr[:, kt, s, ds(c0 // P, w // P), :],
                        start=(kt == 0 and s == 0),
                        stop=(kt == NKT2 - 1 and s == 1),
                    )
                return
            nc.tensor.matmul(
                pt[:, :w],
                bt[:, kt, i, :, :],
                btr[:, kt, :, ds(c0 // P, w // P), :],
                start=(kt == 0),
                stop=(kt == NKT2 - 1),
                perf_mode=DR,
            )

        def gram_warmup():
            """Strips 0 and 1, kt-major across 6 concurrent PSUM chains so the
            Gram accumulation pipelines with the 16 chunked B DMAs."""
            # only the two tiles panels 0/1 need synchronously; the rest
            # of strips 0+1 go into the filler queue so refine-0's PE ops
            # are not stuck behind the whole warmup backlog
            head2 = [(0, 0, FT), (1, P, FT)]
            chains = []
            for (i, c0, w), (pool, tg) in zip(head2, [(gpsum, "gp"),
                                                      (apsum, "ap")]):
                chains.append((i, c0, w, pool.tile([P, FT], f32, tag=tg,
                                                   name="gw")))
            for kt in range(NKT2):
                for (i, c0, w, pt) in chains:
                    gram_mm_chain(pt, i, c0, w, kt)
            for (i, c0, w, pt) in chains:
                nc.scalar.copy(gs[i][:, ds(c0 - i * P, w)], pt[:, :w])
                gram_done[(i, c0)] = True

            def rest01_gen():
                rest = [(0, c0, w) for (c0, w) in _col_tiles(NT, 0)[1:]] + [
                    (1, c0, w) for (c0, w) in _col_tiles(NT - 1, P)[1:]
                ]
                rest = sorted(rest, key=lambda t: t[1])
                for (i, c0, w) in rest:
                    pt = gpsum.tile([P, FT], f32, tag="gp", name="pt")
                    for kt in range(NKT2):
                        gram_mm_chain(pt, i, c0, w, kt)
                        if kt % 2 == 1 and kt < NKT2 - 1:
                            yield True
                    nc.scalar.copy(gs[i][:, ds(c0 - i * P, w)], pt[:, :w])
                    gram_done[(i, c0)] = True
                    yield True

            gramq.append(rest01_gen())

        _neg_state = [0]

        # ---------- software-pipeline state ----------
        from collections import deque

        gram_done = {}     # (strip, c0) -> emitted
        gramq = deque()
        tailq = {0: deque(), 1: deque()}
        emitted = {}       # (m, panel, tix) -> TRSM tile emitted
        pre_state = {0: None, 1: None}
        blk1_state = {0: None, 1: None}
        nblks_of = {0: nblk, 1: NT}
        _ptiles = {}
        for _i in range(nblk):
            _ptiles[(0, _i)] = _panel_tiles(nblk - _i, _i * P)
        for _i in range(NT):
            _ptiles[(1, _i)] = _panel_tiles(NT - _i, _i * P)

        def _gtile_of(i, col):
            return i * P + ((col - i * P) // FT) * FT

        def tile_ix_of(m, j, col):
            for tix, (c0, w) in enumerate(_ptiles[(m, j)]):
                if c0 <= col < c0 + w:
                    return tix
            raise AssertionError((m, j, col))

        def gram_gen(i):
            for (c0, w) in _col_tiles(NT - i, i * P):
                pt = gpsum.tile([P, FT], f32, tag="gp", name="pt")
                for kt in range(NKT2):
                    gram_mm_chain(pt, i, c0, w, kt)
                    if kt % 2 == 1 and kt < NKT2 - 1:
                        yield True
                nc.scalar.copy(gs[i][:, ds(c0 - i * P, w)], pt[:, :w])
                gram_done[(i, c0)] = True
                yield True

        def gram_master():
            if not gramq:
                return False
            g = gramq[0]
            try:
                return next(g)
            except StopIteration:
                gramq.popleft()
                return True

        def pull_gram_until(i, c0):
            guard = 0
            while (i, c0) not in gram_done:
                assert gramq, ("gram starved", i, c0)
                gram_master()
                guard += 1
                assert guard < 100000

        def tail_master(m):
            if not tailq[m]:
                return False
            g = tailq[m][0]
            try:
                return next(g)
            except StopIteration:
                tailq[m].popleft()
                return True

        def pull_tail_until(m, key):
            guard = 0
            while key not in emitted:
                assert tailq[m], ("tail starved", m, key)
                tail_master(m)
                guard += 1
                assert guard < 100000

        def diag_pre_vec(m, i):
            """Mask/I prep of gs diag block for panel i (no Schur terms)."""
            gsl = gs[i][:, ds(0, P)]
            pre = ppool.tile([P, P], bf16, tag=f"pre{m}", name="pre")
            if m == 1:
                nc.vector.tensor_add(pre, gsl, I128)
            else:
                tmp = ppool.tile([P, P], f32, tag="mtmp", name="mtmp")
                nc.gpsimd.tensor_mul(tmp, gsl, mrowrep[:, ds(i * P, P)])
                nc.gpsimd.tensor_scalar_mul(tmp, tmp, mcol[:, ds(i, 1)])
                nc.gpsimd.tensor_add(pre, tmp, dfix_all[:, i, :])
            return pre

        def diag_ap_mm(apt, m, n, j, start, stop=False):
            nc.tensor.matmul(
                apt,
                ub[(m, j)][:, ds((n - j) * P, P)],
                ub[(m, j)][:, ds((n - j) * P, P)],
                start=start,
                stop=stop,
            )

        def start_pre(m, n):
            if m == 0:
                return  # m0 chains are short; emitted force-closed at close
            pre_state[m] = {"n": n, "j": 0, "apt": None, "pre": None,
                            "started": False}

        def advance_pre(m, budget=6, force=False):
            st = pre_state[m]
            if st is None:
                return None
            n = st["n"]
            if st["pre"] is None:
                gc = _gtile_of(n, n * P)
                if (n, gc) not in gram_done:
                    if force:
                        pull_gram_until(n, gc)
                    else:
                        return False
                st["pre"] = diag_pre_vec(m, n)
                st["apt"] = dpsum.tile([P, P], f32, tag="dp", name="dp")
                nc.tensor.matmul(st["apt"], INEGB, st["pre"],
                                 start=True, stop=False)
                st["started"] = True
                budget -= 1
            while st["j"] <= n - 2:
                j = st["j"]
                key = (m, j, tile_ix_of(m, j, n * P))
                if key not in emitted:
                    if force:
                        pull_tail_until(m, key)
                    else:
                        return False
                diag_ap_mm(st["apt"], m, n, j, not st["started"])
                st["started"] = True
                st["j"] += 1
                budget -= 1
                if budget <= 0 and not force:
                    return True
            return None  # ready for close

        def close_diag(m, i, cx):
            sblk, sb = cx["sblk"], cx["sb"]
            if i == 0:
                gsl = gs[0][:, ds(0, P)]
                if m == 1:
                    nc.vector.tensor_add(sblk, gsl, I128)
                else:
                    tmp = ppool.tile([P, P], f32, tag="mtmp", name="mtmp")
                    nc.vector.tensor_mul(tmp, gsl, mrowrep[:, ds(0, P)])
                    nc.vector.tensor_scalar_mul(tmp, tmp, mcol[:, ds(0, 1)])
                    nc.vector.tensor_add(sblk, tmp, dfix_all[:, 0, :])
            elif m == 1:
                advance_pre(m, force=True)
                st = pre_state[m]
                assert st is not None and st["n"] == i
                diag_ap_mm(st["apt"], m, i, i - 1, not st["started"], stop=True)
                cx["sneg"] = st["apt"]
                nc.scalar.mul(sb, st["apt"], -1.0)
                pre_state[m] = None
                return
            else:
                gc = _gtile_of(i, i * P)
                pull_gram_until(i, gc)
                for j in range(i - 1):
                    pull_tail_until(0, (0, j, tile_ix_of(0, j, i * P)))
                pre = diag_pre_vec(0, i)
                apw = wpsum.tile([P, FT], f32, tag="w", name="apw")
                nc.tensor.matmul(apw[:, :P], INEGB, pre, start=True, stop=False)
                for j in range(i):
                    diag_ap_mm(apw[:, :P], 0, i, j, False, stop=(j == i - 1))
                cx["sneg"] = apw[:, :P]
                nc.scalar.mul(sb, apw[:, :P], -1.0)
                return
            (nc.scalar.copy if m == 1 else nc.gpsimd.tensor_copy)(sb, sblk)

        def new_panel(m, i):
            wblk = nblks_of[m] - i
            return {
                "i": i,
                "tiles": _ptiles[(m, i)],
                "strip": spool.tile([P, wblk * P], bf16, tag="strip", name="strip"),
                "sblk": rpool.tile([P, P], f32, tag="sblk", name="sblk"),
                "sb": rpool.tile([P, P], bf16, tag="sb", name="sb"),
                "sneg": None,
            }

        def start_blk1(m, i, cx):
            blk1_state[m] = None
            if len(cx["tiles"]) < 2:
                return
            if i == 0:
                if m == 0:
                    nc.vector.tensor_mul(
                        cx["strip"][:, ds(P, P)], gs[0][:, ds(P, P)],
                        mrowrep[:, ds(P, P)],
                    )
                return
            if m == 0:
                blk1_state[m] = {"i": i, "cx": cx, "forced": True}
                return
            blk1_state[m] = {"i": i, "cx": cx, "j": 0, "apt": None,
                             "pre1": None, "prepped": False, "started": False,
                             "forced": False}

        def advance_blk1(m, budget=6, force=False):
            st = blk1_state[m]
            if st is None or st.get("forced"):
                return None
            i = st["i"]
            cx = st["cx"]
            c0, _w = cx["tiles"][1]
            if not st["prepped"]:
                gc = _gtile_of(i, c0)
                if (i, gc) not in gram_done:
                    if force:
                        pull_gram_until(i, gc)
                    else:
                        return False
                st["apt"] = dpsum.tile([P, P], f32, tag="dp1", name="dp1")
                nc.tensor.matmul(
                    st["apt"], INEGB, gs[i][:, ds(c0 - i * P, P)],
                    start=True, stop=False,
                )
                st["started"] = True
                st["prepped"] = True
                budget -= 1
            while st["j"] <= i - 2:
                j = st["j"]
                k1 = (m, j, tile_ix_of(m, j, i * P))
                k2 = (m, j, tile_ix_of(m, j, c0))
                if k1 not in emitted or k2 not in emitted:
                    if force:
                        pull_tail_until(m, k1)
                        pull_tail_until(m, k2)
                    else:
                        return False
                nc.tensor.matmul(
                    st["apt"],
                    ub[(m, j)][:, ds((i - j) * P, P)],
                    ub[(m, j)][:, ds(c0 - j * P, P)],
                    start=not st["started"],
                    stop=False,
                )
                st["started"] = True
                st["j"] += 1
                budget -= 1
                if budget <= 0 and not force:
                    return True
            return None  # ready for blk1_finish

        def blk1_finish(m):
            st = blk1_state[m]
            if st is None:
                return
            i = st["i"]
            cx = st["cx"]
            c0, _w = cx["tiles"][1]
            dst = cx["strip"][:, ds(c0 - i * P, P)]
            if st.get("forced"):
                gc = _gtile_of(i, c0)
                pull_gram_until(i, gc)
                for j in range(i):
                    if j < i - 1:
                        pull_tail_until(m, (m, j, tile_ix_of(m, j, i * P)))
                    pull_tail_until(m, (m, j, tile_ix_of(m, j, c0)))
                p1 = ppool.tile([P, P], bf16, tag="p1", name="p1")
                nc.gpsimd.tensor_mul(
                    p1, gs[i][:, ds(c0 - i * P, P)], mrowrep[:, ds(c0, P)]
                )
                apw = wpsum.tile([P, FT], f32, tag="w", name="apw")
                nc.tensor.matmul(apw[:, :P], INEGB, p1, start=True, stop=False)
                for j in range(i):
                    nc.tensor.matmul(
                        apw[:, :P],
                        ub[(m, j)][:, ds((i - j) * P, P)],
                        ub[(m, j)][:, ds(c0 - j * P, P)],
                        start=False,
                        stop=(j == i - 1),
                    )
                nc.scalar.mul(dst, apw[:, :P], -1.0)
                blk1_state[m] = None
                return
            advance_blk1(m, force=True)
            key = (m, i - 1, tile_ix_of(m, i - 1, c0))
            pull_tail_until(m, key)
            nc.tensor.matmul(
                st["apt"],
                ub[(m, i - 1)][:, ds(P, P)],
                ub[(m, i - 1)][:, ds(c0 - (i - 1) * P, P)],
                start=not st["started"],
                stop=True,
            )
            nc.scalar.mul(dst, st["apt"], -1.0)
            blk1_state[m] = None

        def refine_gen(m, i, cx):
            """Pivot-block factor; yields at cross-engine handoffs so filler
            matmuls can be emitted between dependent steps.  m1's elementwise
            ops ride DVE, m0's ride Pool (reduces are DVE-only)."""
            E = nc.vector if m == 1 else nc.gpsimd
            sb = cx["sb"]
            sneg = cx["sneg"]
            ssrc = sneg if sneg is not None else cx["sblk"]
            sgn = -1.0 if sneg is not None else 1.0
            dcol = dstore[:, m, ds(i, 1)]
            scr0 = rpool.tile([P, P], f32, tag="scr0", name="scr0")
            nc.vector.tensor_mul(scr0, ssrc, INEGF if sneg is not None else I128)
            nc.vector.tensor_reduce(dcol, scr0, AX, OP.add)
            rinv = vpool.tile([P, 1], f32, tag="rinv", name="rinv")
            nc.vector.reciprocal(rinv, dcol)
            yt = rpool.tile([P, P], f32, tag="yt", name="yt")
            nc.vector.tensor_mul(yt, ssrc, STRILN if sneg is not None else STRIL)
            yu = rpool.tile([P, P], f32, tag="yu", name="yu")
            nc.vector.tensor_mul(yu, ssrc, STRIUN if sneg is not None else STRIU)
            yield
            rt_ps = wpsum.tile([P, FT], f32, tag="w", name="rt_ps")
            nc.tensor.transpose(rt_ps[:1, :P], rinv, I128)
            rcol = vpool.tile([P, 1], f32, tag="rcol", name="rcol")
            nc.scalar.sqrt(rcol, rinv)
            rrow = vpool.tile([1, P], bf16, tag="rrow", name="rrow")
            nc.scalar.sqrt(rrow, rt_ps[:1, :P])
            yield
            q_ps = wpsum.tile([P, FT], f32, tag="w", name="q_ps")
            nc.tensor.matmul(q_ps[:, :P], rrow, rrow, start=True, stop=True)
            x1 = rpool.tile([P, P], bf16, tag="x1", name="x1")
            nc.vector.tensor_mul(x1, yu, q_ps[:, :P])
            x1t = rpool.tile([P, P], bf16, tag="x1t", name="x1t")
            nc.vector.tensor_mul(x1t, yt, q_ps[:, :P])
            yield
            # x2_ps accumulates X1@X1 - X1 + I entirely on PE
            x2_ps = wpsum.tile([P, FT], f32, tag="w", name="x2_ps")
            nc.tensor.matmul(x2_ps[:, :P], x1t, x1, start=True, stop=False)
            nc.tensor.matmul(x2_ps[:, :P], INEGB, x1, start=False, stop=False)
            nc.tensor.matmul(x2_ps[:, :P], I128b, I128b, start=False, stop=True)
            wfac = rpool.tile([P, P], bf16, tag="wfac", name="wfac")
            if m == 1:
                nc.vector.tensor_scalar_mul(wfac, x2_ps[:, :P], rcol)
            else:
                nc.scalar.activation(
                    wfac, x2_ps[:, :P], AF.Copy, scale=rcol)
            yield
            last = len(cx["tiles"]) == 1
            wt = None
            if not last:
                wt_ps = wpsum.tile([P, FT * 2], bf16, tag="w", name="wt_ps")
                nc.tensor.transpose(wt_ps[:, :P], wfac, I128b)
                wt = rpool.tile([P, P], bf16, tag="wt", name="wt")
                nc.scalar.copy(wt, wt_ps[:, :P])
            sw_ps = wpsum.tile([P, FT], f32, tag="w", name="sw_ps")
            nc.tensor.matmul(sw_ps[:, :P], sb, wfac, start=True, stop=True)
            swt = rpool.tile([P, P], bf16, tag="swt", name="swt")
            nc.scalar.copy(swt, sw_ps[:, :P])
            yield
            # fpi_ps accumulates W^T S W - I on PE: result is F itself
            fpi_ps = wpsum.tile([P, FT], f32, tag="w", name="fpi_ps")
            nc.tensor.matmul(fpi_ps[:, :P], wfac, swt, start=True, stop=False)
            nc.tensor.matmul(fpi_ps[:, :P], INEGB, I128b, start=False, stop=True)
            ff = rpool.tile([P, P], bf16, tag="ff", name="ff")
            nc.scalar.copy(ff, fpi_ps[:, :P])
            if not last:
                fs = rpool.tile([P, P], bf16, tag="fs", name="fs")
                E.tensor_scalar_mul(fs, ff, -0.5)
            yield
            if not last:
                wh_ps = wpsum.tile([P, FT], f32, tag="w", name="wh_ps")
                nc.tensor.matmul(wh_ps[:, :P], wt, fs, start=True, stop=True)
                what = rpool.tile([P, P], bf16, tag="what", name="what")
                nc.vector.tensor_add(what, wh_ps[:, :P], wfac)
                cx["what"] = what
                if m == 0:
                    whatm = rpool.tile([P, P], bf16, tag="whatm", name="whatm")
                    nc.gpsimd.tensor_scalar_mul(whatm, what, mcol[:, ds(i, 1)])
                    cx["whatm"] = whatm
            # logdet trace series, from bf16 copies, off the What chain
            trf = vpool.tile([P, 1], f32, tag="trf", name="trf")
            scr1 = rpool.tile([P, P], f32, tag="scr1", name="scr1")
            nc.gpsimd.tensor_mul(scr1, ff, I128)
            nc.vector.tensor_reduce(trf, scr1, AX, OP.add)
            trf2 = vpool.tile([P, 1], f32, tag="trf2", name="trf2")
            scr2 = rpool.tile([P, P], f32, tag="scr2", name="scr2")
            nc.gpsimd.tensor_mul(scr2, ff, ff)
            nc.vector.tensor_reduce(trf2, scr2, AX, OP.add)
            t1 = vpool.tile([P, 1], f32, tag="t1", name="t1")
            nc.vector.tensor_scalar(
                out=t1, in0=trf2, scalar1=-0.5, scalar2=None, op0=OP.mult
            )
            nc.vector.tensor_add(t1, t1, trf)
            nc.vector.tensor_add(acc[:, ds(m, 1)], acc[:, ds(m, 1)], t1)

        def trsm_tile(m, i, cx, tix):
            c0, w = cx["tiles"][tix]
            if m == 1 and i == 0 and tix > 0:
                rhs = gs[0][:, ds(c0, w)]
            elif tix == 0:
                rhs = cx["sb"]
            else:
                rhs = cx["strip"][:, ds(c0 - i * P, w)]
            lhs = cx["what"] if tix == 0 else cx.get("whatm", cx["what"])
            tp = wpsum.tile([P, FT], f32, tag="w", name="tp")
            nc.tensor.matmul(tp[:, :w], lhs, rhs, start=True, stop=True)
            dst = ub[(m, i)][:, ds(c0 - i * P, w)]
            if tix <= 1:
                nc.vector.tensor_copy(dst, tp[:, :w])
            else:
                copy_rr(dst, tp[:, :w])

        def trsm_head(m, i, cx):
            if len(cx["tiles"]) > 1:
                trsm_tile(m, i, cx, 1)
                emitted[(m, i, 1)] = True

        def tail_gen(m, i, cx):
            tiles = cx["tiles"]
            strip = cx["strip"]
            for tix in range(2, len(tiles)):
                c0, w = tiles[tix]
                for col in (c0, c0 + w - 1):
                    gc = _gtile_of(i, col)
                    while (i, gc) not in gram_done:
                        yield False
                if not (m == 1 and i == 0):
                    gsl = gs[i][:, ds(c0 - i * P, w)]
                    dst = strip[:, ds(c0 - i * P, w)]
                    if i > 0:
                        if m == 0:
                            tmpm = spool.tile([P, FT], bf16, tag="ptmp",
                                              name="tmpm")
                            tt_eng().tensor_mul(
                                tmpm[:, :w], gsl, mrowrep[:, ds(c0, w)]
                            )
                            neg_src = tmpm[:, :w]
                        else:
                            neg_src = gsl
                        ap = apsum.tile([P, FT], f32, tag="ap", name="ap")
                        nc.tensor.matmul(
                            ap[:, :w], INEGB, neg_src, start=True, stop=False
                        )
                        for j in range(i):
                            nc.tensor.matmul(
                                ap[:, :w],
                                ub[(m, j)][:, ds((i - j) * P, P)],
                                ub[(m, j)][:, ds(c0 - j * P, w)],
                                start=False,
                                stop=(j == i - 1),
                            )
                            if j % 2 == 1 and j < i - 1:
                                yield True
                        nonlocal_ns = _neg_state[0] = _neg_state[0] + 1
                        if nonlocal_ns % 3 == 0:
                            nc.scalar.mul(dst, ap[:, :w], -1.0)
                        else:
                            nc.vector.tensor_scalar(
                                out=dst, in0=ap[:, :w], scalar1=-1.0,
                                scalar2=None, op0=OP.mult,
                            )
                    else:
                        tt_eng().tensor_mul(dst, gsl, mrowrep[:, ds(c0, w)])
                    yield True
                trsm_tile(m, i, cx, tix)
                emitted[(m, i, tix)] = True
                yield True

        # ---- software-pipelined emission --------------------------------
        gram_warmup()

        for t in range(NT):
            panels = [(1, t)]
            j0 = t - OFF
            if 0 <= j0 < nblk:
                panels.append((0, j0))
            cxs = {}
            gens = []
            live = []
            # m1 closes and primes its refine first; m0's boundary work then
            # lands in m1's early refine window (PE idle during sqrt/ttr).
            for (m, i) in panels:
                cxs[m] = new_panel(m, i)
                close_diag(m, i, cxs[m])
                start_blk1(m, i, cxs[m])
                g = refine_gen(m, i, cxs[m])
                next(g)
                gens.append(g)
                live.append(g)
            for (m, i) in panels:
                if i + 1 < nblks_of[m]:
                    start_pre(m, i + 1)
            if t + 2 < NT:
                gramq.append(gram_gen(t + 2))

            def mk(fn, *a):
                return lambda: fn(*a)

            tasks = [mk(tail_master, 1), mk(tail_master, 0), mk(gram_master)]
            for (m, i) in panels:
                if blk1_state[m] is not None:
                    tasks.append(mk(advance_blk1, m))
                if pre_state[m] is not None:
                    tasks.append(mk(advance_pre, m))
            fi = 0
            while live:
                for g in list(live):
                    try:
                        next(g)
                    except StopIteration:
                        live.remove(g)
                prog = 0
                attempts = 0
                while tasks and prog < 3 and attempts < 2 * len(tasks):
                    tk = tasks[fi % len(tasks)]
                    fi += 1
                    attempts += 1
                    r = tk()
                    if r is None:
                        tasks.remove(tk)
                    elif r:
                        prog += 1
            for (m, i) in panels:
                blk1_finish(m)
                trsm_head(m, i, cxs[m])
                if len(cxs[m]["tiles"]) > 2:
                    tailq[m].append(tail_gen(m, i, cxs[m]))

        guard = 0
        while tailq[0] or tailq[1] or gramq:
            p1 = tail_master(1)
            p0 = tail_master(0)
            pg = gram_master()
            guard = 0 if (p1 or p0 or pg) else guard + 1
            assert guard < 1000, "drain deadlock"

        # -------- final: batched Ln(d), partition-sum via matmul ------
        lnall = vpool.tile([P, 2, NT], f32, tag="lnall", name="lnall")
        nc.scalar.activation(
            lnall.rearrange("p a b -> p (a b)"),
            dstore.rearrange("p a b -> p (a b)"), AF.Ln,
        )
        ln0 = vpool.tile([P, 1], f32, tag="ln0", name="ln0")
        nc.vector.tensor_reduce(ln0, lnall[:, 0, :], AX, OP.add)
        ln1 = vpool.tile([P, 1], f32, tag="ln1", name="ln1")
        nc.vector.tensor_reduce(ln1, lnall[:, 1, :], AX, OP.add)
        accd = vpool.tile([P, 1], f32, tag="accd", name="accd")
        nc.vector.tensor_sub(accd, acc[:, 0:1], acc[:, 1:2])
        nc.vector.tensor_add(accd, accd, ln0)
        nc.vector.tensor_sub(accd, accd, ln1)
        ones = vpool.tile([P, 1], f32, tag="ones", name="ones")
        nc.vector.memset(ones, 1.0)
        r_ps = wpsum.tile([P, FT], f32, tag="w", name="r_ps")
        nc.tensor.matmul(r_ps[:1, :1], accd, ones, start=True, stop=True)
        res = vpool.tile([1, 1], f32, tag="res", name="res")
        nc.vector.tensor_copy(res, r_ps[:1, :1])
        nc.sync.dma_start(out_d[:, :], res)

    nc.finalize()
    return nc


def kernel(x, B):
    """Full inputs -> full output. x: [8, 2048] int32, B: [2000, 2048] f32."""
    from concourse.bass_utils import run_bass_kernel_spmd

    bs, n = x.shape
    k = B.shape[0]
    assert n == N and bs == 8

    B8 = B.astype(ml_dtypes.float8_e4m3fn)
    percore = []
    nblk = 1
    for c in range(bs):
        m = x[c] == 1
        sel = np.where(m)[0]
        unsel = np.where(~m)[0]
        n_c = len(sel)
        percore.append((np.concatenate([sel, unsel]), n_c))
        nblk = max(nblk, -(-n_c // P))

    if nblk not in _CACHE:
        _CACHE[nblk] = _build(nblk)
    nc = _CACHE[nblk]

    in_maps = []
    for perm, n_c in percore:
        bqc = np.zeros((N, N), dtype=ml_dtypes.float8_e4m3fn)
        bqc[:k, :] = B8[:, perm]
        # [p, kt2, blk, slab, c] layout, flattened per partition row
        bqc = np.ascontiguousarray(
            bqc.reshape(NKT2, 2, P, NT, P)
            .transpose(2, 0, 3, 1, 4)
            .reshape(P, NKT2 * NT * 2 * P)
        )
        idx = np.arange(nblk * P)
        mcol = np.ascontiguousarray(
            (idx.reshape(nblk, P).T < n_c).astype(np.float32)
        )
        mrow = np.ascontiguousarray(
            np.broadcast_to(
                (idx < n_c).astype(ml_dtypes.bfloat16), (P, nblk * P)
            )
        )
        in_maps.append({"bq": bqc, "mcol": mcol, "mrow": mrow})
    global _last_in_maps
    _last_in_maps = in_maps
    res = run_bass_kernel_spmd(nc, in_maps, core_ids=list(range(bs)))
    out = np.array([r["out"][0, 0] for r in res.results], dtype=np.float32)
    return out


# revision 43
# speedup vs baseline: 1.1306x; 1.0091x over previous
"""Trainium2 Bass kernel for nn_DPP: batched masked-Gram logdet minus shared
normalizer logdet.

out[i] = logdet(G * m_i m_i^T + diag(1-m_i)) - logdet(G + I),  G = B^T B

Sharding: data-parallel over the batch dim of x (one sample per NeuronCore,
B replicated). Host-side trick: each core gets B with its sample's SELECTED
columns permuted to the front, so the masked logdet is the logdet of the
LEADING ~n_sel block of the permuted Gram G' (logdet(G+I) is permutation
invariant), and one Gram serves both factorizations.

Device algorithm (per core):
  - G' = Bq^T Bq upper-triangle strips via fp8(e4m3) DoubleRow matmuls
    (fp32 PSUM accum, 2x PE throughput), B loaded in 16 chunked DMAs that
    overlap with the first Gram strips' accumulation chains.
  - Two interleaved left-looking blocked Cholesky factorizations (U-form,
    128-wide panels): A0 = leading-nblk-block masked G' (+ identity pad on
    partial blocks), A1 = G' + I.  A0's panels are OFFSET to pair with A1's
    tail panels.  Panels are software-pipelined: each panel's diagonal
    Schur chain is pre-accumulated (open PSUM group) during the PREVIOUS
    panel's refine, TRSM emits the diag+block1 tiles first so the next
    diagonal closes with a single matmul, and the remaining TRSM tiles /
    trailing accumulations / Gram strips fill the refine latency.
  - Each 128x128 diagonal pivot S is handled matmul-only ("refine" scheme):
      d = diag(S); r = 1/sqrt(d); q = r r^T
      X1 = striu(S) * q; X1T = stril(S) * q      (striu(DSD) = striu(S)*q)
      W = diag(r) (I - X1 + X1@X1)               (approx inv-chol factor)
      F = W^T S W - I                            (small)
      logdet(S) = sum(ln d) + tr F - tr F^2/2 + tr F^3/3
      What = W + W(-F/2 + 3F^2/8)                (What What^T ~ S^{-1})
    Panel: U_strip = What^T @ strip; trailing Schur updates use U (bf16).
    All ln d are batched into one ACT Ln at the end.
"""

import numpy as np
import ml_dtypes

P = 128
N = 2048           # matrix dim (= n columns of B)
NT = N // P        # 16 column tiles
NKT = 16           # contraction tiles (B rows padded 2000 -> 2048)
NKT2 = 8           # fp8 DoubleRow pairs
FT = 512           # free-dim tile for wide matmuls

_CACHE = {}
_last_in_maps = None
_PLAIN_GRAM = False  # CoreSim-only fallback (interp lacks 4D DoubleRow)


def _col_tiles(width_blocks, base_col, diag_first=False):
    """Gram tiling: split cols into <=512 tiles from the strip start."""
    tiles = []
    c = base_col
    end = base_col + width_blocks * P
    if diag_first:
        tiles.append((c, P))
        c += P
    while c < end:
        w = min(FT, end - c)
        tiles.append((c, w))
        c += w
    return tiles


def _panel_tiles(width_blocks, base_col):
    """Panel tiling: [diag P][block1 P][pad to abs 512 grid][512 grid...].
    block1 is split out so the next panel's diagonal Schur term needs only
    the first two TRSM tiles; the rest is 512-grid aligned."""
    end = base_col + width_blocks * P
    tiles = [(base_col, P)]
    c = base_col + P
    if c < end:
        tiles.append((c, P))
        c += P
    if c < end and c % FT:
        w = min(FT - c % FT, end - c)
        tiles.append((c, w))
        c += w
    while c < end:
        w = min(FT, end - c)
        tiles.append((c, w))
        c += w
    return tiles


def _build(nblk):
    import concourse.bass as bass
    import concourse.bacc as bacc
    import concourse.mybir as mybir
    from concourse.bass import ds, ts
    from concourse.masks import (
        make_identity,
        make_upper_triangular,
        make_lower_triangular,
    )
    from concourse.tile import TileContext
    from contextlib import ExitStack

    f32 = mybir.dt.float32
    bf16 = mybir.dt.bfloat16
    f8 = mybir.dt.float8e4
    AF = mybir.ActivationFunctionType
    OP = mybir.AluOpType
    PSUM = bass.MemorySpace.PSUM
    AX = mybir.AxisListType.X
    DR = mybir.MatmulPerfMode.DoubleRow
    OFF = NT - nblk  # m0 panel j runs at step t = j + OFF

    nc = bacc.Bacc()
    # B in block-pair layout [p, kt2, blk, slab, c] flattened per partition:
    # DoubleRow weights slices must be contiguous [P, 2, 128]
    bq = nc.dram_tensor("bq", [P, NKT2 * NT * 2 * P], f8, kind="ExternalInput")
    mcol_d = nc.dram_tensor("mcol", [P, nblk], f32, kind="ExternalInput")
    mrow_d = nc.dram_tensor("mrow", [P, nblk * P], bf16, kind="ExternalInput")
    out_d = nc.dram_tensor("out", [1, 1], f32, kind="ExternalOutput")

    with TileContext(nc) as tc, ExitStack() as stack:
        consts = stack.enter_context(tc.tile_pool(name="consts", bufs=1))
        bpool = stack.enter_context(tc.tile_pool(name="bpool", bufs=1))
        bt = bpool.tile([P, NKT2, NT, 2, P], f8, tag="bt")
        btf = bt.rearrange("p k b s c -> p (k b s c)")
        CH = NT * 2 * P   # one kt2 pair-slab chunk
        CQ = 8 * 2 * P    # first 8 column blocks of a chunk
        # B-load DMAs lead the queue; first-half chunks first so strips
        # 0/1's early tiles (blocks 0-7) land ~8us earlier
        for kt in range(NKT2):
            nc.sync.dma_start(btf[:, ds(kt * CH, CQ)], bq[:, ds(kt * CH, CQ)])
        for kt in range(NKT2):
            nc.sync.dma_start(
                btf[:, ds(kt * CH + CQ, CH - CQ)], bq[:, ds(kt * CH + CQ, CH - CQ)]
            )
        btr = bt.rearrange("p k b s c -> p k s b c")
        I128 = consts.tile([P, P], f32, tag="i128")
        make_identity(nc, I128)
        I128b = consts.tile([P, P], bf16, tag="i128b")
        nc.vector.tensor_copy(I128b, I128)
        INEGB = consts.tile([P, P], bf16, tag="inegb")
        nc.vector.tensor_scalar(
            out=INEGB, in0=I128, scalar1=-1.0, scalar2=None, op0=OP.mult
        )
        STRIU = consts.tile([P, P], f32, tag="striu")
        make_upper_triangular(nc, STRIU, val=1.0, diag=False)
        STRIL = consts.tile([P, P], f32, tag="stril")
        make_lower_triangular(nc, STRIL, val=1.0, diag=False)
        STRIUN = consts.tile([P, P], f32, tag="striun")
        make_upper_triangular(nc, STRIUN, val=-1.0, diag=False)
        STRILN = consts.tile([P, P], f32, tag="striln")
        make_lower_triangular(nc, STRILN, val=-1.0, diag=False)
        INEGF = consts.tile([P, P], f32, tag="inegf")
        nc.vector.tensor_scalar(
            out=INEGF, in0=I128, scalar1=-1.0, scalar2=None, op0=OP.mult
        )
        mcol = consts.tile([P, nblk], f32, tag="mcol")
        nc.sync.dma_start(mcol, mcol_d[:, :])
        mrowrep = consts.tile([P, nblk * P], bf16, tag="mrowrep")
        nc.sync.dma_start(mrowrep, mrow_d[:, :])
        acc = consts.tile([P, 2], f32, tag="acc")
        nc.vector.memset(acc, 0.0)
        dstore = consts.tile([P, 2, NT], f32, tag="dstore")
        nc.vector.memset(dstore.rearrange("p a b -> p (a b)"), 1.0)
        onem_all = consts.tile([P, nblk], f32, tag="onem_all")
        nc.vector.tensor_scalar(
            out=onem_all, in0=mcol, scalar1=-1.0, scalar2=1.0,
            op0=OP.mult, op1=OP.add,
        )
        dfix_all = consts.tile([P, nblk, P], f32, tag="dfix_all")
        for i in range(nblk):
            nc.vector.tensor_scalar_mul(dfix_all[:, i, :], I128, onem_all[:, ds(i, 1)])

        gs = []  # gs[i]: [P, (NT-i)*P] bf16, absolute cols i*128..2048
        for i in range(NT):
            gs.append(consts.tile([P, (NT - i) * P], bf16, tag=f"gs{i}", name=f"gs{i}"))
        ub = {}  # panels of the two factorizations (m0: nblk-wide, m1: full)
        for i in range(nblk):
            ub[(0, i)] = consts.tile(
                [P, (nblk - i) * P], bf16, tag=f"ub0_{i}", name=f"ub0_{i}"
            )
        for i in range(NT):
            ub[(1, i)] = consts.tile(
                [P, (NT - i) * P], bf16, tag=f"ub1_{i}", name=f"ub1_{i}"
            )

        gpsum = stack.enter_context(tc.tile_pool(name="gram_psum", bufs=1, space=PSUM))
        spool = stack.enter_context(tc.tile_pool(name="strip_pool", bufs=4))
        ppool = stack.enter_context(tc.tile_pool(name="pre_pool", bufs=6))
        rpool = stack.enter_context(tc.tile_pool(name="ref_pool", bufs=4))
        vpool = stack.enter_context(tc.tile_pool(name="vec_pool", bufs=4))
        apsum = stack.enter_context(tc.tile_pool(name="acc_psum", bufs=2, space=PSUM))
        wpsum = stack.enter_context(tc.tile_pool(name="work_psum", bufs=3, space=PSUM))
        dpsum = stack.enter_context(tc.tile_pool(name="diag_psum", bufs=1, space=PSUM))


        # round-robin engine pickers for balanced elementwise work
        _tt_state = 0
        _cp_state = 0

        def tt_eng():
            nonlocal _tt_state
            _tt_state += 1
            return (nc.vector, nc.gpsimd)[_tt_state % 2]

        def copy_rr(out, in_):
            nonlocal _cp_state
            _cp_state += 1
            if _cp_state % 2 == 0:
                nc.scalar.copy(out, in_)
            else:
                nc.vector.tensor_copy(out, in_)

        def gram_mm_chain(pt, i, c0, w, kt):
            if _PLAIN_GRAM:
                for s in range(2):
                    nc.tensor.matmul(
                        pt[:, :w],
                        bt[:, kt, i, s, :],
                        btr[:, kt, s, ds(c0 // P, w // P), :],
                        start=(kt == 0 and s == 0),
                        stop=(kt == NKT2 - 1 and s == 1),
                    )
                return
            nc.tensor.matmul(
                pt[:, :w],
                bt[:, kt, i, :, :],
                btr[:, kt, :, ds(c0 // P, w // P), :],
                start=(kt == 0),
                stop=(kt == NKT2 - 1),
                perf_mode=DR,
            )

        def gram_warmup():
            """Strips 0 and 1, kt-major across 6 concurrent PSUM chains so the
            Gram accumulation pipelines with the 16 chunked B DMAs."""
            # only the two tiles panels 0/1 need synchronously; the rest
            # of strips 0+1 go into the filler queue so refine-0's PE ops
            # are not stuck behind the whole warmup backlog
            head2 = [(0, 0, FT), (1, P, FT)]
            chains = []
            for (i, c0, w), (pool, tg) in zip(head2, [(gpsum, "gp"),
                                                      (apsum, "ap")]):
                chains.append((i, c0, w, pool.tile([P, FT], f32, tag=tg,
                                                   name="gw")))
            for kt in range(NKT2):
                for (i, c0, w, pt) in chains:
                    gram_mm_chain(pt, i, c0, w, kt)
            for (i, c0, w, pt) in chains:
                nc.scalar.copy(gs[i][:, ds(c0 - i * P, w)], pt[:, :w])
                gram_done[(i, c0)] = True

            def rest01_gen():
                rest = [(0, c0, w) for (c0, w) in _col_tiles(NT, 0)[1:]] + [
                    (1, c0, w) for (c0, w) in _col_tiles(NT - 1, P)[1:]
                ]
                rest = sorted(rest, key=lambda t: t[1])
                for (i, c0, w) in rest:
                    pt = gpsum.tile([P, FT], f32, tag="gp", name="pt")
                    for kt in range(NKT2):
                        gram_mm_chain(pt, i, c0, w, kt)
                        if kt % 2 == 1 and kt < NKT2 - 1:
                            yield True
                    nc.scalar.copy(gs[i][:, ds(c0 - i * P, w)], pt[:, :w])
                    gram_done[(i, c0)] = True
                    yield True

            gramq.append(rest01_gen())

        _neg_state = [0]

        # ---------- software-pipeline state ----------
        from collections import deque

        gram_done = {}     # (strip, c0) -> emitted
        gramq = deque()
        tailq = {0: deque(), 1: deque()}
        emitted = {}       # (m, panel, tix) -> TRSM tile emitted
        pre_state = {0: None, 1: None}
        blk1_state = {0: None, 1: None}
        nblks_of = {0: nblk, 1: NT}
        _ptiles = {}
        for _i in range(nblk):
            _ptiles[(0, _i)] = _panel_tiles(nblk - _i, _i * P)
        for _i in range(NT):
            _ptiles[(1, _i)] = _panel_tiles(NT - _i, _i * P)

        def _gtile_of(i, col):
            return i * P + ((col - i * P) // FT) * FT

        def tile_ix_of(m, j, col):
            for tix, (c0, w) in enumerate(_ptiles[(m, j)]):
                if c0 <= col < c0 + w:
                    return tix
            raise AssertionError((m, j, col))

        def gram_gen(i):
            for (c0, w) in _col_tiles(NT - i, i * P):
                pt = gpsum.tile([P, FT], f32, tag="gp", name="pt")
                for kt in range(NKT2):
                    gram_mm_chain(pt, i, c0, w, kt)
                    if kt % 2 == 1 and kt < NKT2 - 1:
                        yield True
                nc.scalar.copy(gs[i][:, ds(c0 - i * P, w)], pt[:, :w])
                gram_done[(i, c0)] = True
                yield True

        def gram_master():
            if not gramq:
                return False
            g = gramq[0]
            try:
                return next(g)
            except StopIteration:
                gramq.popleft()
                return True

        def pull_gram_until(i, c0):
            guard = 0
            while (i, c0) not in gram_done:
                assert gramq, ("gram starved", i, c0)
                gram_master()
                guard += 1
                assert guard < 100000

        def tail_master(m):
            if not tailq[m]:
                return False
            g = tailq[m][0]
            try:
                return next(g)
            except StopIteration:
                tailq[m].popleft()
                return True

        def pull_tail_until(m, key):
            guard = 0
            while key not in emitted:
                assert tailq[m], ("tail starved", m, key)
                tail_master(m)
                guard += 1
                assert guard < 100000

        def diag_pre_vec(m, i):
            """Mask/I prep of gs diag block for panel i (no Schur terms)."""
            gsl = gs[i][:, ds(0, P)]
            pre = ppool.tile([P, P], bf16, tag=f"pre{m}", name="pre")
            if m == 1:
                nc.vector.tensor_add(pre, gsl, I128)
            else:
                tmp = ppool.tile([P, P], f32, tag="mtmp", name="mtmp")
                nc.gpsimd.tensor_mul(tmp, gsl, mrowrep[:, ds(i * P, P)])
                nc.gpsimd.tensor_scalar_mul(tmp, tmp, mcol[:, ds(i, 1)])
                nc.gpsimd.tensor_add(pre, tmp, dfix_all[:, i, :])
            return pre

        def diag_ap_mm(apt, m, n, j, start, stop=False):
            nc.tensor.matmul(
                apt,
                ub[(m, j)][:, ds((n - j) * P, P)],
                ub[(m, j)][:, ds((n - j) * P, P)],
                start=start,
                stop=stop,
            )

        def start_pre(m, n):
            if m == 0:
                return  # m0 chains are short; emitted force-closed at close
            pre_state[m] = {"n": n, "j": 0, "apt": None, "pre": None,
                            "started": False}

        def advance_pre(m, budget=6, force=False):
            st = pre_state[m]
            if st is None:
                return None
            n = st["n"]
            if st["pre"] is None:
                gc = _gtile_of(n, n * P)
                if (n, gc) not in gram_done:
                    if force:
                        pull_gram_until(n, gc)
                    else:
                        return False
                st["pre"] = diag_pre_vec(m, n)
                st["apt"] = dpsum.tile([P, P], f32, tag="dp", name="dp")
                nc.tensor.matmul(st["apt"], INEGB, st["pre"],
                                 start=True, stop=False)
                st["started"] = True
                budget -= 1
            while st["j"] <= n - 2:
                j = st["j"]
                key = (m, j, tile_ix_of(m, j, n * P))
                if key not in emitted:
                    if force:
                        pull_tail_until(m, key)
                    else:
                        return False
                diag_ap_mm(st["apt"], m, n, j, not st["started"])
                st["started"] = True
                st["j"] += 1
                budget -= 1
                if budget <= 0 and not force:
                    return True
            return None  # ready for close

        def close_diag(m, i, cx):
            sblk, sb = cx["sblk"], cx["sb"]
            if i == 0:
                gsl = gs[0][:, ds(0, P)]
                if m == 1:
                    nc.vector.tensor_add(sblk, gsl, I128)
                else:
                    tmp = ppool.tile([P, P], f32, tag="mtmp", name="mtmp")
                    nc.vector.tensor_mul(tmp, gsl, mrowrep[:, ds(0, P)])
                    nc.vector.tensor_scalar_mul(tmp, tmp, mcol[:, ds(0, 1)])
                    nc.vector.tensor_add(sblk, tmp, dfix_all[:, 0, :])
            elif m == 1:
                advance_pre(m, force=True)
                st = pre_state[m]
                assert st is not None and st["n"] == i
                diag_ap_mm(st["apt"], m, i, i - 1, not st["started"], stop=True)
                cx["sneg"] = st["apt"]
                nc.scalar.mul(sb, st["apt"], -1.0)
                pre_state[m] = None
                return
            else:
                gc = _gtile_of(i, i * P)
                pull_gram_until(i, gc)
                for j in range(i - 1):
                    pull_tail_until(0, (0, j, tile_ix_of(0, j, i * P)))
                pre = diag_pre_vec(0, i)
                apw = wpsum.tile([P, FT], f32, tag="w", name="apw")
                nc.tensor.matmul(apw[:, :P], INEGB, pre, start=True, stop=False)
                for j in range(i):
                    diag_ap_mm(apw[:, :P], 0, i, j, False, stop=(j == i - 1))
                cx["sneg"] = apw[:, :P]
                nc.scalar.mul(sb, apw[:, :P], -1.0)
                return
            (nc.scalar.copy if m == 1 else nc.gpsimd.tensor_copy)(sb, sblk)

        def new_panel(m, i):
            wblk = nblks_of[m] - i
            return {
                "i": i,
                "tiles": _ptiles[(m, i)],
                "strip": spool.tile([P, wblk * P], bf16, tag="strip", name="strip"),
                "sblk": rpool.tile([P, P], f32, tag="sblk", name="sblk"),
                "sb": rpool.tile([P, P], bf16, tag="sb", name="sb"),
                "sneg": None,
            }

        def start_blk1(m, i, cx):
            blk1_state[m] = None
            if len(cx["tiles"]) < 2:
                return
            if i == 0:
                if m == 0:
                    nc.vector.tensor_mul(
                        cx["strip"][:, ds(P, P)], gs[0][:, ds(P, P)],
                        mrowrep[:, ds(P, P)],
                    )
                return
            if m == 0:
                blk1_state[m] = {"i": i, "cx": cx, "forced": True}
                return
            blk1_state[m] = {"i": i, "cx": cx, "j": 0, "apt": None,
                             "pre1": None, "prepped": False, "started": False,
                             "forced": False}

        def advance_blk1(m, budget=6, force=False):
            st = blk1_state[m]
            if st is None or st.get("forced"):
                return None
            i = st["i"]
            cx = st["cx"]
            c0, _w = cx["tiles"][1]
            if not st["prepped"]:
                gc = _gtile_of(i, c0)
                if (i, gc) not in gram_done:
                    if force:
                        pull_gram_until(i, gc)
                    else:
                        return False
                st["apt"] = dpsum.tile([P, P], f32, tag="dp1", name="dp1")
                nc.tensor.matmul(
                    st["apt"], INEGB, gs[i][:, ds(c0 - i * P, P)],
                    start=True, stop=False,
                )
                st["started"] = True
                st["prepped"] = True
                budget -= 1
            while st["j"] <= i - 2:
                j = st["j"]
                k1 = (m, j, tile_ix_of(m, j, i * P))
                k2 = (m, j, tile_ix_of(m, j, c0))
                if k1 not in emitted or k2 not in emitted:
                    if force:
                        pull_tail_until(m, k1)
                        pull_tail_until(m, k2)
                    else:
                        return False
                nc.tensor.matmul(
                    st["apt"],
                    ub[(m, j)][:, ds((i - j) * P, P)],
                    ub[(m, j)][:, ds(c0 - j * P, P)],
                    start=not st["started"],
                    stop=False,
                )
                st["started"] = True
                st["j"] += 1
                budget -= 1
                if budget <= 0 and not force:
                    return True
            return None  # ready for blk1_finish

        def blk1_finish(m):
            st = blk1_state[m]
            if st is None:
                return
            i = st["i"]
            cx = st["cx"]
            c0, _w = cx["tiles"][1]
            dst = cx["strip"][:, ds(c0 - i * P, P)]
            if st.get("forced"):
                gc = _gtile_of(i, c0)
                pull_gram_until(i, gc)
                for j in range(i):
                    if j < i - 1:
                        pull_tail_until(m, (m, j, tile_ix_of(m, j, i * P)))
                    pull_tail_until(m, (m, j, tile_ix_of(m, j, c0)))
                p1 = ppool.tile([P, P], bf16, tag="p1", name="p1")
                nc.gpsimd.tensor_mul(
                    p1, gs[i][:, ds(c0 - i * P, P)], mrowrep[:, ds(c0, P)]
                )
                apw = wpsum.tile([P, FT], f32, tag="w", name="apw")
                nc.tensor.matmul(apw[:, :P], INEGB, p1, start=True, stop=False)
                for j in range(i):
                    nc.tensor.matmul(
                        apw[:, :P],
                        ub[(m, j)][:, ds((i - j) * P, P)],
                        ub[(m, j)][:, ds(c0 - j * P, P)],
                        start=False,
                        stop=(j == i - 1),
                    )
                nc.scalar.mul(dst, apw[:, :P], -1.0)
                blk1_state[m] = None
                return
            advance_blk1(m, force=True)
            key = (m, i - 1, tile_ix_of(m, i - 1, c0))
            pull_tail_until(m, key)
            nc.tensor.matmul(
                st["apt"],
                ub[(m, i - 1)][:, ds(P, P)],
                ub[(m, i - 1)][:, ds(c0 - (i - 1) * P, P)],
                start=not st["started"],
                stop=True,
            )
            nc.scalar.mul(dst, st["apt"], -1.0)
            blk1_state[m] = None

        def refine_gen(m, i, cx):
            """Pivot-block factor; yields at cross-engine handoffs so filler
            matmuls can be emitted between dependent steps.  m1's elementwise
            ops ride DVE, m0's ride Pool (reduces are DVE-only)."""
            E = nc.vector if m == 1 else nc.gpsimd
            sb = cx["sb"]
            sneg = cx["sneg"]
            ssrc = sneg if sneg is not None else cx["sblk"]
            sgn = -1.0 if sneg is not None else 1.0
            dcol = dstore[:, m, ds(i, 1)]
            scr0 = rpool.tile([P, P], f32, tag="scr0", name="scr0")
            nc.vector.tensor_mul(scr0, ssrc, INEGF if sneg is not None else I128)
            nc.vector.tensor_reduce(dcol, scr0, AX, OP.add)
            rinv = vpool.tile([P, 1], f32, tag="rinv", name="rinv")
            nc.vector.reciprocal(rinv, dcol)
            yt = rpool.tile([P, P], f32, tag="yt", name="yt")
            nc.vector.tensor_mul(yt, ssrc, STRILN if sneg is not None else STRIL)
            yu = rpool.tile([P, P], f32, tag="yu", name="yu")
            nc.vector.tensor_mul(yu, ssrc, STRIUN if sneg is not None else STRIU)
            yield
            rt_ps = wpsum.tile([P, FT], f32, tag="w", name="rt_ps")
            nc.tensor.transpose(rt_ps[:1, :P], rinv, I128)
            rcol = vpool.tile([P, 1], f32, tag="rcol", name="rcol")
            nc.scalar.sqrt(rcol, rinv)
            rrow = vpool.tile([1, P], bf16, tag="rrow", name="rrow")
            nc.scalar.sqrt(rrow, rt_ps[:1, :P])
            yield
            q_ps = wpsum.tile([P, FT], f32, tag="w", name="q_ps")
            nc.tensor.matmul(q_ps[:, :P], rrow, rrow, start=True, stop=True)
            x1 = rpool.tile([P, P], bf16, tag="x1", name="x1")
            nc.vector.tensor_mul(x1, yu, q_ps[:, :P])
            x1t = rpool.tile([P, P], bf16, tag="x1t", name="x1t")
            nc.vector.tensor_mul(x1t, yt, q_ps[:, :P])
            yield
            # x2_ps accumulates X1@X1 - X1 + I entirely on PE
            x2_ps = wpsum.tile([P, FT], f32, tag="w", name="x2_ps")
            nc.tensor.matmul(x2_ps[:, :P], x1t, x1, start=True, stop=False)
            nc.tensor.matmul(x2_ps[:, :P], INEGB, x1, start=False, stop=False)
            nc.tensor.matmul(x2_ps[:, :P], I128b, I128b, start=False, stop=True)
            wfac = rpool.tile([P, P], bf16, tag="wfac", name="wfac")
            if m == 1:
                nc.vector.tensor_scalar_mul(wfac, x2_ps[:, :P], rcol)
            else:
                nc.scalar.activation(
                    wfac, x2_ps[:, :P], AF.Copy, scale=rcol)
            yield
            last = len(cx["tiles"]) == 1
            wt = None
            if not last:
                wt_ps = wpsum.tile([P, FT * 2], bf16, tag="w", name="wt_ps")
                nc.tensor.transpose(wt_ps[:, :P], wfac, I128b)
                wt = rpool.tile([P, P], bf16, tag="wt", name="wt")
                nc.scalar.copy(wt, wt_ps[:, :P])
            sw_ps = wpsum.tile([P, FT], f32, tag="w", name="sw_ps")
            nc.tensor.matmul(sw_ps[:, :P], sb, wfac, start=True, stop=True)
            swt = rpool.tile([P, P], bf16, tag="swt", name="swt")
            nc.scalar.copy(swt, sw_ps[:, :P])
            yield
            # fpi_ps accumulates W^T S W - I on PE: result is F itself
            fpi_ps = wpsum.tile([P, FT], f32, tag="w", name="fpi_ps")
            nc.tensor.matmul(fpi_ps[:, :P], wfac, swt, start=True, stop=False)
            nc.tensor.matmul(fpi_ps[:, :P], INEGB, I128b, start=False, stop=True)
            ff = rpool.tile([P, P], bf16, tag="ff", name="ff")
            nc.scalar.copy(ff, fpi_ps[:, :P])
            if not last:
                fs = rpool.tile([P, P], bf16, tag="fs", name="fs")
                E.tensor_scalar_mul(fs, ff, -0.5)
            yield
            if not last:
                wh_ps = wpsum.tile([P, FT], f32, tag="w", name="wh_ps")
                nc.tensor.matmul(wh_ps[:, :P], wt, fs, start=True, stop=True)
                what = rpool.tile([P, P], bf16, tag="what", name="what")
                nc.vector.tensor_add(what, wh_ps[:, :P], wfac)
                cx["what"] = what
                if m == 0:
                    whatm = rpool.tile([P, P], bf16, tag="whatm", name="whatm")
                    nc.gpsimd.tensor_scalar_mul(whatm, what, mcol[:, ds(i, 1)])
                    cx["whatm"] = whatm
            # logdet trace series, from bf16 copies, off the What chain
            trf = vpool.tile([P, 1], f32, tag="trf", name="trf")
            scr1 = rpool.tile([P, P], f32, tag="scr1", name="scr1")
            nc.gpsimd.tensor_mul(scr1, ff, I128)
            nc.vector.tensor_reduce(trf, scr1, AX, OP.add)
            trf2 = vpool.tile([P, 1], f32, tag="trf2", name="trf2")
            scr2 = rpool.tile([P, P], f32, tag="scr2", name="scr2")
            nc.gpsimd.tensor_mul(scr2, ff, ff)
            nc.vector.tensor_reduce(trf2, scr2, AX, OP.add)
            t1 = vpool.tile([P, 1], f32, tag="t1", name="t1")
            nc.vector.tensor_scalar(
                out=t1, in0=trf2, scalar1=-0.5, scalar2=None, op0=OP.mult
            )
            nc.vector.tensor_add(t1, t1, trf)
            nc.vector.tensor_add(acc[:, ds(m, 1)], acc[:, ds(m, 1)], t1)

        def trsm_tile(m, i, cx, tix):
            c0, w = cx["tiles"][tix]
            if m == 1 and i == 0 and tix > 0:
                rhs = gs[0][:, ds(c0, w)]
            elif tix == 0:
                rhs = cx["sb"]
            else:
                rhs = cx["strip"][:, ds(c0 - i * P, w)]
            lhs = cx["what"] if tix == 0 else cx.get("whatm", cx["what"])
            tp = wpsum.tile([P, FT], f32, tag="w", name="tp")
            nc.tensor.matmul(tp[:, :w], lhs, rhs, start=True, stop=True)
            dst = ub[(m, i)][:, ds(c0 - i * P, w)]
            if tix <= 1:
                nc.vector.tensor_copy(dst, tp[:, :w])
            else:
                copy_rr(dst, tp[:, :w])

        def trsm_head(m, i, cx):
            if len(cx["tiles"]) > 1:
                trsm_tile(m, i, cx, 1)
                emitted[(m, i, 1)] = True

        def tail_gen(m, i, cx):
            tiles = cx["tiles"]
            strip = cx["strip"]
            for tix in range(2, len(tiles)):
                c0, w = tiles[tix]
                for col in (c0, c0 + w - 1):
                    gc = _gtile_of(i, col)
                    while (i, gc) not in gram_done:
                        yield False
                if not (m == 1 and i == 0):
                    gsl = gs[i][:, ds(c0 - i * P, w)]
                    dst = strip[:, ds(c0 - i * P, w)]
                    if i > 0:
                        if m == 0:
                            tmpm = spool.tile([P, FT], bf16, tag="ptmp",
                                              name="tmpm")
                            tt_eng().tensor_mul(
                                tmpm[:, :w], gsl, mrowrep[:, ds(c0, w)]
                            )
                            neg_src = tmpm[:, :w]
                        else:
                            neg_src = gsl
                        ap = apsum.tile([P, FT], f32, tag="ap", name="ap")
                        nc.tensor.matmul(
                            ap[:, :w], INEGB, neg_src, start=True, stop=False
                        )
                        for j in range(i):
                            nc.tensor.matmul(
                                ap[:, :w],
                                ub[(m, j)][:, ds((i - j) * P, P)],
                                ub[(m, j)][:, ds(c0 - j * P, w)],
                                start=False,
                                stop=(j == i - 1),
                            )
                            if j % 2 == 1 and j < i - 1:
                                yield True
                        nonlocal_ns = _neg_state[0] = _neg_state[0] + 1
                        if nonlocal_ns % 3 == 0:
                            nc.scalar.mul(dst, ap[:, :w], -1.0)
                        else:
                            nc.vector.tensor_scalar(
                                out=dst, in0=ap[:, :w], scalar1=-1.0,
                                scalar2=None, op0=OP.mult,
                            )
                    else:
                        tt_eng().tensor_mul(dst, gsl, mrowrep[:, ds(c0, w)])
                    yield True
                trsm_tile(m, i, cx, tix)
                emitted[(m, i, tix)] = True
                yield True

        # ---- software-pipelined emission --------------------------------
        gram_warmup()

        for t in range(NT):
            panels = [(1, t)]
            j0 = t - OFF
            if 0 <= j0 < nblk:
                panels.append((0, j0))
            cxs = {}
            gens = []
            live = []
            # m1 closes and primes its refine first; m0's boundary work then
            # lands in m1's early refine window (PE idle during sqrt/ttr).
            for (m, i) in panels:
                cxs[m] = new_panel(m, i)
                close_diag(m, i, cxs[m])
                start_blk1(m, i, cxs[m])
                g = refine_gen(m, i, cxs[m])
                next(g)
                gens.append(g)
                live.append(g)
            for (m, i) in panels:
                if i + 1 < nblks_of[m]:
                    start_pre(m, i + 1)
            if t + 2 < NT:
                gramq.append(gram_gen(t + 2))

            def mk(fn, *a):
                return lambda: fn(*a)

            tasks = [mk(tail_master, 1), mk(tail_master, 0), mk(gram_master)]
            for (m, i) in panels:
                if blk1_state[m] is not None:
                    tasks.append(mk(advance_blk1, m))
                if pre_state[m] is not None:
                    tasks.append(mk(advance_pre, m))
            fi = 0
            while live:
                for g in list(live):
                    try:
                        next(g)
                    except StopIteration:
                        live.remove(g)
                prog = 0
                attempts = 0
                while tasks and prog < 3 and attempts < 2 * len(tasks):
                    tk = tasks[fi % len(tasks)]
                    fi += 1
                    attempts += 1
                    r = tk()
                    if r is None:
                        tasks.remove(tk)
                    elif r:
                        prog += 1
            for (m, i) in panels:
                blk1_finish(m)
                trsm_head(m, i, cxs[m])
                if len(cxs[m]["tiles"]) > 2:
                    tailq[m].append(tail_gen(m, i, cxs[m]))

        guard = 0
        while tailq[0] or tailq[1] or gramq:
            p1 = tail_master(1)
            p0 = tail_master(0)
            pg = gram_master()
            guard = 0 if (p1 or p0 or pg) else guard + 1
            assert guard < 1000, "drain deadlock"

        # -------- final: batched Ln(d), partition-sum via matmul ------
        lnall = vpool.tile([P, 2, NT], f32, tag="lnall", name="lnall")
        nc.scalar.activation(
            lnall.rearrange("p a b -> p (a b)"),
            dstore.rearrange("p a b -> p (a b)"), AF.Ln,
        )
        ln0 = vpool.tile([P, 1], f32, tag="ln0", name="ln0")
        nc.vector.tensor_reduce(ln0, lnall[:, 0, :], AX, OP.add)
        ln1 = vpool.tile([P, 1], f32, tag="ln1", name="ln1")
        nc.vector.tensor_reduce(ln1, lnall[:, 1, :], AX, OP.add)
        accd = vpool.tile([P, 1], f32, tag="accd", name="accd")
        nc.vector.tensor_sub(accd, acc[:, 0:1], acc[:, 1:2])
        nc.vector.tensor_add(accd, accd, ln0)
        nc.vector.tensor_sub(accd, accd, ln1)
        ones = vpool.tile([P, 1], f32, tag="ones", name="ones")
        nc.vector.memset(ones, 1.0)
        r_ps = wpsum.tile([P, FT], f32, tag="w", name="r_ps")
        nc.tensor.matmul(r_ps[:1, :1], accd, ones, start=True, stop=True)
        res = vpool.tile([1, 1], f32, tag="res", name="res")
        nc.vector.tensor_copy(res, r_ps[:1, :1])
        nc.sync.dma_start(out_d[:, :], res)

    nc.finalize()
    return nc


def kernel(x, B):
    """Full inputs -> full output. x: [8, 2048] int32, B: [2000, 2048] f32."""
    from concourse.bass_utils import run_bass_kernel_spmd

    bs, n = x.shape
    k = B.shape[0]
    assert n == N and bs == 8

    B8 = B.astype(ml_dtypes.float8_e4m3fn)
    percore = []
    nblk = 1
    for c in range(bs):
        m = x[c] == 1
        sel = np.where(m)[0]
        unsel = np.where(~m)[0]
        n_c = len(sel)
        percore.append((np.concatenate([sel, unsel]), n_c))
        nblk = max(nblk, -(-n_c // P))

    if nblk not in _CACHE:
        _CACHE[nblk] = _build(nblk)
    nc = _CACHE[nblk]

    in_maps = []
    for perm, n_c in percore:
        bqc = np.zeros((N, N), dtype=ml_dtypes.float8_e4m3fn)
        bqc[:k, :] = B8[:, perm]
        # [p, kt2, blk, slab, c] layout, flattened per partition row
        bqc = np.ascontiguousarray(
            bqc.reshape(NKT2, 2, P, NT, P)
            .transpose(2, 0, 3, 1, 4)
            .reshape(P, NKT2 * NT * 2 * P)
        )
        idx = np.arange(nblk * P)
        mcol = np.ascontiguousarray(
            (idx.reshape(nblk, P).T < n_c).astype(np.float32)
        )
        mrow = np.ascontiguousarray(
            np.broadcast_to(
                (idx < n_c).astype(ml_dtypes.bfloat16), (P, nblk * P)
            )
        )
        in_maps.append({"bq": bqc, "mcol": mcol, "mrow": mrow})
    global _last_in_maps
    _last_in_maps = in_maps
    res = run_bass_kernel_spmd(nc, in_maps, core_ids=list(range(bs)))
    out = np.array([r["out"][0, 0] for r in res.results], dtype=np.float32)
    return out
